# revision 1
# baseline (speedup 1.0000x reference)
"""GAT (2x GATConv + global_mean_pool + MLP) on 8 Trainium2 NeuronCores.

Strategy (sharding_hint: 1D node partition, replicated weights):
  - dst nodes partitioned 8 ways (1250/core, padded to 10 tiles of 128 slots);
    edges sorted by dst, grouped into per-(core,tile) blocks of 128.
  - Layer 1 aggregates x[src] (128 wide) instead of h[src] (1024 wide):
    sum_e ex*(x W1) == (sum_e ex*x) W1 per head -> 8x less gather traffic.
    Attention logits via fused vectors V = W @ a (alpha = x @ V).
  - Segment softmax-sum via one-hot indicator matmuls on the tensor engine
    (PSUM accumulation); normalization after projection (commutes per head).
  - Only exchange: AllGather of per-core [1280, 528] fp16 packed table
    (h2p = elu(out1) @ W2, plus alpha2_src bitcast) + [1280, 8] f32 alpha2_dst,
    and a tiny AllReduce of pooled per-graph sums. MLP replicated.
All float math in f32 on device (fp16 only for the exchanged table).
"""
import os
import sys
import numpy as np

for _p in ("/opt/trn_rl_repo",):
    if os.path.isdir(_p) and _p not in sys.path:
        sys.path.insert(0, _p)

N = 10000
B = 16
NCORES = 8
P = 128
NPC = 1250                  # nodes per core
TPC = 10                    # dst tiles per core
NPAD = 10112                # 79 * 128
NTILES_A = 79
NEG = 0.2
L2ROWS = NCORES * TPC * P   # 10240

_PROGRAM_CACHE = {}
LAST_PROFILE = {}
DEBUG_DUMPS = False


def _preprocess(edge_index, batch):
    src = np.concatenate([np.asarray(edge_index[0]), np.arange(N)]).astype(np.int64)
    dst = np.concatenate([np.asarray(edge_index[1]), np.arange(N)]).astype(np.int64)
    order = np.argsort(dst, kind='stable')
    src, dst = src[order], dst[order]

    core_of = dst // NPC
    local = dst - core_of * NPC
    tile_of = local // P
    seg_of = (local - tile_of * P).astype(np.float32)

    counts = np.zeros((NCORES, TPC), dtype=np.int64)
    np.add.at(counts, (core_of, tile_of), 1)
    bt = int(np.ceil(counts.max() / P))
    bt = max(bt, 1)

    src_m = np.zeros((NCORES, TPC, P, bt), dtype=np.int32)
    dst_m = np.zeros((NCORES, TPC, P, bt), dtype=np.int32)
    seg_m = np.full((NCORES, TPC, P, bt), -1.0, dtype=np.float32)

    flat_group = core_of * TPC + tile_of
    grp_start = np.searchsorted(flat_group, np.arange(NCORES * TPC), 'left')
    rank = np.arange(len(flat_group)) - grp_start[flat_group]
    blk = rank // P
    part = rank % P
    co = core_of.astype(np.int64)
    ti = tile_of.astype(np.int64)
    src_m[co, ti, part, blk] = src.astype(np.int32)
    dst_m[co, ti, part, blk] = dst.astype(np.int32)
    seg_m[co, ti, part, blk] = seg_of

    # L2 table rows: node n lives at core*1280 + (n - core*1250)
    node = np.arange(N, dtype=np.int64)
    cn = node // NPC
    l2row = (cn * TPC * P + (node - cn * NPC)).astype(np.int32)
    srcl2_m = l2row[src_m.reshape(-1)].reshape(src_m.shape)
    dstl2_m = l2row[dst_m.reshape(-1)].reshape(dst_m.shape)

    batch = np.asarray(batch).astype(np.int64)
    gid = np.full((NCORES, TPC, P), -1.0, dtype=np.float32)
    for c in range(NCORES):
        lo = c * NPC
        hi = min(lo + NPC, N)
        vals = batch[lo:hi].astype(np.float32)
        g = gid[c].reshape(-1)
        g[:hi - lo] = vals

    cnt = np.zeros(B, np.float32)
    np.add.at(cnt, batch, 1.0)
    recip_cnt16 = (1.0 / np.maximum(cnt, 1.0)).astype(np.float32).reshape(16, 1)

    return dict(bt=bt, src_m=src_m, dst_m=dst_m, seg_m=seg_m,
                srcl2_m=srcl2_m, dstl2_m=dstl2_m, gid=gid,
                recip_cnt16=recip_cnt16)


def _build_program(bt, upto='full', repeat=1):
    import concourse.bacc as bacc
    import concourse.bass as bass
    import concourse.mybir as mybir
    import concourse.tile as tile
    from concourse.masks import make_identity

    f32 = mybir.dt.float32
    f16 = mybir.dt.float16
    i32 = mybir.dt.int32
    AF = mybir.ActivationFunctionType
    OP = mybir.AluOpType
    IOA = bass.IndirectOffsetOnAxis

    _ORDER = ['none', 'A', 'B', 'AG', 'E', 'AR', 'F', 'full']
    def _inc(s):
        return _ORDER.index(upto if upto != 'full' else 'F') >= _ORDER.index(s)

    nc = bacc.Bacc("TRN2", target_bir_lowering=False, debug=False,
                   enable_asserts=False, num_devices=NCORES)

    # ---------------- inputs ----------------
    t_x = nc.dram_tensor("x_pad", [NPAD, P], f32, kind="ExternalInput")
    t_W1 = nc.dram_tensor("W1", [P, 1024], f32, kind="ExternalInput")
    t_V1 = nc.dram_tensor("V1", [P, 16], f32, kind="ExternalInput")
    t_W2 = nc.dram_tensor("W2", [1024, 512], f32, kind="ExternalInput")
    t_V2 = nc.dram_tensor("V2", [1024, 16], f32, kind="ExternalInput")
    t_b1r = nc.dram_tensor("b1_rep", [P, 1024], f32, kind="ExternalInput")
    t_b2r = nc.dram_tensor("b2_rep", [P, 512], f32, kind="ExternalInput")
    t_iota128 = nc.dram_tensor("iota128", [P, P], f32, kind="ExternalInput")
    t_iota16 = nc.dram_tensor("iota16", [P, 16], f32, kind="ExternalInput")
    t_rc16 = nc.dram_tensor("recip_cnt16", [16, 1], f32, kind="ExternalInput")
    t_fc1w = nc.dram_tensor("fc1_w", [512, 32], f32, kind="ExternalInput")
    t_fc1b = nc.dram_tensor("fc1_b", [32, 1], f32, kind="ExternalInput")
    t_fc2w = nc.dram_tensor("fc2_w", [32, 10], f32, kind="ExternalInput")
    t_fc2br = nc.dram_tensor("fc2_b_rep", [16, 10], f32, kind="ExternalInput")
    t_srcm = nc.dram_tensor("src_m", [TPC, P, bt], i32, kind="ExternalInput")
    t_dstm = nc.dram_tensor("dst_m", [TPC, P, bt], i32, kind="ExternalInput")
    t_segm = nc.dram_tensor("seg_m", [TPC, P, bt], f32, kind="ExternalInput")
    t_srcl2 = nc.dram_tensor("srcl2_m", [TPC, P, bt], i32, kind="ExternalInput")
    t_dstl2 = nc.dram_tensor("dstl2_m", [TPC, P, bt], i32, kind="ExternalInput")
    t_gid = nc.dram_tensor("gid_m", [TPC, P], f32, kind="ExternalInput")
    t_chain = nc.dram_tensor("chain", [16, 10], f32, kind="ExternalInput")

    t_out = nc.dram_tensor("out", [16, 10], f32, kind="ExternalOutput")
    dbg = {}
    if DEBUG_DUMPS:
        dbg['xa'] = nc.dram_tensor("dbg_xa", [NPAD, 136], f32, kind="ExternalOutput")
        dbg['a1d'] = nc.dram_tensor("dbg_a1d", [NPAD, 8], f32, kind="ExternalOutput")
        dbg['h2p'] = nc.dram_tensor("dbg_h2p", [TPC * P, 512], f16, kind="ExternalOutput")
        dbg['a2'] = nc.dram_tensor("dbg_a2", [TPC * P, 16], f32, kind="ExternalOutput")
        dbg['pool'] = nc.dram_tensor("dbg_pool", [16, 512], f32, kind="ExternalOutput")
        dbg['poolr'] = nc.dram_tensor("dbg_poolr", [16, 512], f32, kind="ExternalOutput")
        dbg['h2pf'] = nc.dram_tensor("dbg_h2pf", [L2ROWS, 512], f16, kind="ExternalOutput")
        dbg['a2f'] = nc.dram_tensor("dbg_a2f", [L2ROWS, 16], f32, kind="ExternalOutput")

    with tile.TileContext(nc) as tc:
        with (
            tc.tile_pool(name="const", bufs=1) as csb,
            tc.tile_pool(name="dram", bufs=1, space="DRAM") as dr,
        ):
            # DRAM staging
            xa_tab = dr.tile([NPAD, 136], f32)          # [x | alpha1_src]
            a1d_tab = dr.tile([NPAD, 8], f32)           # alpha1_dst
            h2p_loc = dr.tile([TPC * P, 512], f16)
            a2_loc = dr.tile([TPC * P, 16], f32)        # [a2src | a2dst]
            a2s_tab = dr.tile([L2ROWS, 8], f32)
            a2d_tab = dr.tile([L2ROWS, 8], f32)
            pool_in = dr.tile([16, 512], f32)

            # resident constants
            ident = csb.tile([P, P], f32)
            make_identity(nc, ident[:])
            iota128 = csb.tile([P, P], f32)
            nc.sync.dma_start(out=iota128[:], in_=t_iota128[:])
            iota16 = csb.tile([P, 16], f32)
            nc.sync.dma_start(out=iota16[:], in_=t_iota16[:])
            W1sb = csb.tile([P, 1024], f32)
            nc.sync.dma_start(out=W1sb[:], in_=t_W1[:])
            V1sb = csb.tile([P, 16], f32)
            nc.sync.dma_start(out=V1sb[:], in_=t_V1[:])
            W2sb = []
            V2sb = []
            for c in range(8):
                w2c = csb.tile([P, 512], f32, name=f"w2c{c}")
                nc.sync.dma_start(out=w2c[:], in_=t_W2[c * P:(c + 1) * P, :])
                W2sb.append(w2c)
                v2c = csb.tile([P, 16], f32, name=f"v2c{c}")
                nc.sync.dma_start(out=v2c[:], in_=t_V2[c * P:(c + 1) * P, :])
                V2sb.append(v2c)
            b1r = csb.tile([P, 1024], f32)
            nc.sync.dma_start(out=b1r[:], in_=t_b1r[:])
            b2r = csb.tile([P, 512], f32)
            nc.sync.dma_start(out=b2r[:], in_=t_b2r[:])
            rc16 = csb.tile([16, 1], f32)
            nc.sync.dma_start(out=rc16[:], in_=t_rc16[:])
            chain_sb = csb.tile([16, 10], f32)
            nc.sync.dma_start(out=chain_sb[:], in_=t_chain[:])

            # bulk copy x into xa_tab[:, 0:128]
            nc.sync.dma_start(out=xa_tab[:, 0:P], in_=t_x[:])

            for _rep in range(repeat):
                h2p_full = dr.tile([L2ROWS, 512], f16, addr_space="Shared",
                                   name=f"h2p_full{_rep}")
                a2_full = dr.tile([L2ROWS, 16], f32, addr_space="Shared",
                                  name=f"a2_full{_rep}")
                pool_out = dr.tile([16, 512], f32, addr_space="Shared",
                                   name=f"pool_out{_rep}")
                # ---------------- Phase A: alpha1 tables (replicated) ---------
                with (
                    tc.tile_pool(name="pa_sb", bufs=3) as asb,
                    tc.tile_pool(name="pa_ps", bufs=2, space="PSUM") as aps,
                ):
                    for t in range(NTILES_A if _inc('A') else 0):
                        sl = slice(t * P, (t + 1) * P)
                        xt = asb.tile([P, P], f32, name="xt")
                        nc.sync.dma_start(out=xt[:], in_=t_x[sl, :])
                        xT_ps = aps.tile([P, P], f32, name="xT_ps")
                        nc.tensor.transpose(out=xT_ps[:], in_=xt[:], identity=ident[:])
                        xT = asb.tile([P, P], f32, name="xT")
                        nc.vector.tensor_copy(out=xT[:], in_=xT_ps[:])
                        al_ps = aps.tile([P, 16], f32, name="al_ps")
                        nc.tensor.matmul(al_ps[:], lhsT=xT[:], rhs=V1sb[:],
                                         start=True, stop=True)
                        al = asb.tile([P, 16], f32, name="al")
                        nc.vector.tensor_copy(out=al[:], in_=al_ps[:])
                        nc.sync.dma_start(out=xa_tab[sl, P:P + 8], in_=al[:, 0:8])
                        nc.sync.dma_start(out=a1d_tab[sl, :], in_=al[:, 8:16])

                # ---------------- Phase B: L1 aggregation + finalize ----------
                with (
                    tc.tile_pool(name="pb_sb", bufs=4) as bsb,
                    tc.tile_pool(name="pb_big", bufs=3) as bbig,
                    tc.tile_pool(name="pb_fin", bufs=2) as bfin,
                    tc.tile_pool(name="pb_ps", bufs=1, space="PSUM") as bps,
                ):
                    for t in range(TPC):
                        srcm = bsb.tile([P, bt], i32, name="srcm")
                        nc.sync.dma_start(out=srcm[:], in_=t_srcm[t])
                        dstm = bsb.tile([P, bt], i32, name="dstm")
                        nc.sync.dma_start(out=dstm[:], in_=t_dstm[t])
                        segm = bsb.tile([P, bt], f32, name="segm")
                        nc.sync.dma_start(out=segm[:], in_=t_segm[t])

                        axden = bps.tile([P, 1536], f32, name="axden", bufs=1)
                        for b in range(bt):
                            xag = bbig.tile([P, 136], f32, name="xag")
                            nc.gpsimd.indirect_dma_start(
                                out=xag[:], out_offset=None, in_=xa_tab[:],
                                in_offset=IOA(ap=srcm[:, b:b + 1], axis=0))
                            adg = bsb.tile([P, 8], f32, name="adg")
                            nc.gpsimd.indirect_dma_start(
                                out=adg[:], out_offset=None, in_=a1d_tab[:],
                                in_offset=IOA(ap=dstm[:, b:b + 1], axis=0))
                            oh = bbig.tile([P, P], f32, name="oh")
                            nc.vector.tensor_scalar(
                                out=oh[:], in0=iota128[:], scalar1=segm[:, b:b + 1],
                                scalar2=None, op0=OP.is_equal)
                            e = bsb.tile([P, 8], f32, name="e")
                            nc.vector.tensor_add(out=e[:], in0=xag[:, P:P + 8],
                                                 in1=adg[:])
                            es = bsb.tile([P, 8], f32, name="es")
                            nc.vector.tensor_scalar_mul(out=es[:], in0=e[:],
                                                        scalar1=NEG)
                            lr = bsb.tile([P, 8], f32, name="lr")
                            nc.vector.tensor_max(out=lr[:], in0=e[:], in1=es[:])
                            ex = bsb.tile([P, 8], f32, name="ex")
                            nc.scalar.activation(ex[:], lr[:], AF.Exp)
                            msg = bbig.tile([P, 1024], f32, name="msg")
                            msg_v = msg[:].rearrange("p (h c) -> p h c", h=8)
                            xg_b = xag[:, 0:P].unsqueeze(1).broadcast_to([P, 8, P])
                            ex_b = ex[:].unsqueeze(2).broadcast_to([P, 8, P])
                            nc.vector.tensor_tensor(out=msg_v, in0=xg_b, in1=ex_b,
                                                    op=OP.mult)
                            st = (b == 0)
                            sp = (b == bt - 1)
                            nc.tensor.matmul(axden[:, 0:512], lhsT=oh[:],
                                             rhs=msg[:, 0:512], start=st, stop=sp)
                            nc.tensor.matmul(axden[:, 512:1024], lhsT=oh[:],
                                             rhs=msg[:, 512:1024], start=st, stop=sp)
                            nc.tensor.matmul(axden[:, 1024:1032], lhsT=oh[:],
                                             rhs=ex[:], start=st, stop=sp)

                        # ---- finalize tile t
                        den = bsb.tile([P, 8], f32, name="den")
                        nc.vector.tensor_scalar_max(out=den[:],
                                                    in0=axden[:, 1024:1032],
                                                    scalar1=1e-30)
                        rec = bsb.tile([P, 8], f32, name="rec")
                        nc.vector.reciprocal(out=rec[:], in_=den[:])
                        axsb = bfin.tile([P, 1024], f32, name="axsb")
                        nc.vector.tensor_copy(out=axsb[:], in_=axden[:, 0:1024])

                        y = bfin.tile([P, 1024], f32, name="y")
                        for h in range(8):
                            hs = slice(h * P, (h + 1) * P)
                            tps = bps.tile([P, P], f32, name="tps", tag="pp", bufs=3)
                            nc.tensor.transpose(out=tps[:], in_=axsb[:, hs],
                                                identity=ident[:])
                            tsb = bfin.tile([P, P], f32, name="tsb", tag="tsb", bufs=3)
                            nc.vector.tensor_copy(out=tsb[:], in_=tps[:])
                            o1p = bps.tile([P, P], f32, name="o1p", tag="pp", bufs=3)
                            nc.tensor.matmul(o1p[:], lhsT=tsb[:], rhs=W1sb[:, hs],
                                             start=True, stop=True)
                            nc.vector.tensor_scalar_mul(out=y[:, hs], in0=o1p[:],
                                                        scalar1=rec[:, h:h + 1])
                        y2 = bfin.tile([P, 1024], f32, name="y2")
                        nc.vector.tensor_add(out=y2[:], in0=y[:], in1=b1r[:])
                        # elu = (max(y2,0) - 1) + exp(min(y2,0))
                        neg = bfin.tile([P, 1024], f32, name="neg")
                        nc.vector.tensor_scalar_min(out=neg[:], in0=y2[:], scalar1=0.0)
                        en = bfin.tile([P, 1024], f32, name="en")
                        nc.scalar.activation(en[:], neg[:], AF.Exp)
                        pm1 = bfin.tile([P, 1024], f32, name="pm1")
                        nc.vector.tensor_scalar(out=pm1[:], in0=y2[:], scalar1=0.0,
                                                scalar2=-1.0, op0=OP.max, op1=OP.add)
                        e1 = bfin.tile([P, 1024], f32, name="e1")
                        nc.vector.tensor_add(out=e1[:], in0=pm1[:], in1=en[:])

                        h2p_ps = bps.tile([P, 512], f32, name="h2p_ps", bufs=1)
                        a2_ps = bps.tile([P, 16], f32, name="a2_ps", bufs=1)
                        for c in range(8):
                            cs = slice(c * P, (c + 1) * P)
                            tps2 = bps.tile([P, P], f32, name="tps2", tag="pp", bufs=3)
                            nc.tensor.transpose(out=tps2[:], in_=e1[:, cs],
                                                identity=ident[:])
                            tsb2 = bfin.tile([P, P], f32, name="tsb2", tag="tsb", bufs=3)
                            nc.vector.tensor_copy(out=tsb2[:], in_=tps2[:])
                            nc.tensor.matmul(h2p_ps[:], lhsT=tsb2[:], rhs=W2sb[c][:],
                                             start=(c == 0), stop=(c == 7))
                            nc.tensor.matmul(a2_ps[:], lhsT=tsb2[:], rhs=V2sb[c][:],
                                             start=(c == 0), stop=(c == 7))
                        pk = bfin.tile([P, 512], f16, name="pk")
                        nc.vector.tensor_copy(out=pk[:], in_=h2p_ps[:])
                        a2sb = bsb.tile([P, 16], f32, name="a2sb")
                        nc.vector.tensor_copy(out=a2sb[:], in_=a2_ps[:])
                        sl = slice(t * P, (t + 1) * P)
                        nc.sync.dma_start(out=h2p_loc[sl, :], in_=pk[:])
                        nc.sync.dma_start(out=a2_loc[sl, :], in_=a2sb[:])

                # ---------------- AllGather exchange --------------------------
                if _inc('AG'):
                    nc.gpsimd.collective_compute(
                        "AllGather", mybir.AluOpType.bypass,
                        replica_groups=[list(range(NCORES))],
                        ins=[h2p_loc[:].opt()], outs=[h2p_full[:].opt()])
                    nc.gpsimd.collective_compute(
                        "AllGather", mybir.AluOpType.bypass,
                        replica_groups=[list(range(NCORES))],
                        ins=[a2_loc[:].opt()], outs=[a2_full[:].opt()])
                    # split [a2src | a2dst] into separate gather tables
                    nc.sync.dma_start(out=a2s_tab[:], in_=a2_full[:, 0:8])
                    nc.sync.dma_start(out=a2d_tab[:], in_=a2_full[:, 8:16])

                # ---------------- Phase E: L2 aggregation + pool --------------
                with (
                    tc.tile_pool(name="pe_sb", bufs=4) as esb,
                    tc.tile_pool(name="pe_big", bufs=3) as ebig,
                    tc.tile_pool(name="pe_fin", bufs=2) as efin,
                    tc.tile_pool(name="pe_ps", bufs=1, space="PSUM") as eps,
                ):
                    pool_ps = eps.tile([16, 512], f32, name="pool_ps", bufs=1)
                    for t in range(TPC if _inc('E') else 0):
                        srcm = esb.tile([P, bt], i32, name="srcm2")
                        nc.sync.dma_start(out=srcm[:], in_=t_srcl2[t])
                        dstm = esb.tile([P, bt], i32, name="dstm2")
                        nc.sync.dma_start(out=dstm[:], in_=t_dstl2[t])
                        segm = esb.tile([P, bt], f32, name="segm2")
                        nc.sync.dma_start(out=segm[:], in_=t_segm[t])

                        ahden = eps.tile([P, 520], f32, name="ahden", bufs=2)
                        for b in range(bt):
                            hg = ebig.tile([P, 512], f16, name="hg")
                            nc.gpsimd.indirect_dma_start(
                                out=hg[:], out_offset=None, in_=h2p_full[:],
                                in_offset=IOA(ap=srcm[:, b:b + 1], axis=0))
                            asg = esb.tile([P, 8], f32, name="asg2")
                            nc.gpsimd.indirect_dma_start(
                                out=asg[:], out_offset=None, in_=a2s_tab[:],
                                in_offset=IOA(ap=srcm[:, b:b + 1], axis=0))
                            adg = esb.tile([P, 8], f32, name="adg2")
                            nc.gpsimd.indirect_dma_start(
                                out=adg[:], out_offset=None, in_=a2d_tab[:],
                                in_offset=IOA(ap=dstm[:, b:b + 1], axis=0))
                            oh = ebig.tile([P, P], f32, name="oh2")
                            nc.vector.tensor_scalar(
                                out=oh[:], in0=iota128[:], scalar1=segm[:, b:b + 1],
                                scalar2=None, op0=OP.is_equal)
                            e = esb.tile([P, 8], f32, name="e2")
                            nc.vector.tensor_add(out=e[:], in0=asg[:], in1=adg[:])
                            es = esb.tile([P, 8], f32, name="es2")
                            nc.vector.tensor_scalar_mul(out=es[:], in0=e[:],
                                                        scalar1=NEG)
                            lr = esb.tile([P, 8], f32, name="lr2")
                            nc.vector.tensor_max(out=lr[:], in0=e[:], in1=es[:])
                            ex = esb.tile([P, 8], f32, name="ex2")
                            nc.scalar.activation(ex[:], lr[:], AF.Exp)
                            msg = ebig.tile([P, 512], f32, name="msg2")
                            msg_v = msg[:].rearrange("p (h c) -> p h c", h=8)
                            hg_v = hg[:, 0:512].rearrange("p (h c) -> p h c", h=8)
                            ex_b = ex[:].unsqueeze(2).broadcast_to([P, 8, 64])
                            nc.vector.tensor_tensor(out=msg_v, in0=hg_v, in1=ex_b,
                                                    op=OP.mult)
                            st = (b == 0)
                            sp = (b == bt - 1)
                            nc.tensor.matmul(ahden[:, 0:512], lhsT=oh[:],
                                             rhs=msg[:], start=st, stop=sp)
                            nc.tensor.matmul(ahden[:, 512:520], lhsT=oh[:],
                                             rhs=ex[:], start=st, stop=sp)

                        # ---- finalize tile t
                        den = esb.tile([P, 8], f32, name="den2")
                        nc.vector.tensor_scalar_max(out=den[:],
                                                    in0=ahden[:, 512:520],
                                                    scalar1=1e-30)
                        rec = esb.tile([P, 8], f32, name="rec2")
                        nc.vector.reciprocal(out=rec[:], in_=den[:])
                        y = efin.tile([P, 512], f32, name="yl2")
                        for h in range(8):
                            hs = slice(h * 64, (h + 1) * 64)
                            nc.vector.tensor_scalar_mul(out=y[:, hs],
                                                        in0=ahden[:, hs],
                                                        scalar1=rec[:, h:h + 1])
                        y2 = efin.tile([P, 512], f32, name="y2l2")
                        nc.vector.tensor_add(out=y2[:], in0=y[:], in1=b2r[:])
                        neg = efin.tile([P, 512], f32, name="negl2")
                        nc.vector.tensor_scalar_min(out=neg[:], in0=y2[:], scalar1=0.0)
                        en = efin.tile([P, 512], f32, name="enl2")
                        nc.scalar.activation(en[:], neg[:], AF.Exp)
                        pm1 = efin.tile([P, 512], f32, name="pm1l2")
                        nc.vector.tensor_scalar(out=pm1[:], in0=y2[:], scalar1=0.0,
                                                scalar2=-1.0, op0=OP.max, op1=OP.add)
                        e2t = efin.tile([P, 512], f32, name="e2t")
                        nc.vector.tensor_add(out=e2t[:], in0=pm1[:], in1=en[:])

                        gidt = esb.tile([P, 1], f32, name="gidt")
                        nc.sync.dma_start(out=gidt[:], in_=t_gid[t, :, None])
                        gone = esb.tile([P, 16], f32, name="gone")
                        nc.vector.tensor_scalar(out=gone[:], in0=iota16[:],
                                                scalar1=gidt[:, 0:1], scalar2=None,
                                                op0=OP.is_equal)
                        nc.tensor.matmul(pool_ps[:], lhsT=gone[:], rhs=e2t[:],
                                         start=(t == 0), stop=(t == TPC - 1))

                    # pool -> dram bounce
                    if _inc('E'):
                        pool_sb = esb.tile([16, 512], f32, name="pool_sb")
                        nc.vector.tensor_copy(out=pool_sb[:], in_=pool_ps[:])
                        nc.sync.dma_start(out=pool_in[:], in_=pool_sb[:])

                if _inc('AR'):
                    nc.gpsimd.collective_compute(
                        "AllReduce", mybir.AluOpType.add,
                        replica_groups=[list(range(NCORES))],
                        ins=[pool_in[:].opt()], outs=[pool_out[:].opt()])

                if DEBUG_DUMPS:
                    nc.sync.dma_start(out=dbg['xa'][:], in_=xa_tab[:])
                    nc.sync.dma_start(out=dbg['a1d'][:], in_=a1d_tab[:])
                    nc.sync.dma_start(out=dbg['h2p'][:], in_=h2p_loc[:])
                    nc.sync.dma_start(out=dbg['a2'][:], in_=a2_loc[:])
                    nc.sync.dma_start(out=dbg['pool'][:], in_=pool_in[:])
                    nc.sync.dma_start(out=dbg['poolr'][:], in_=pool_out[:])
                    nc.sync.dma_start(out=dbg['h2pf'][:], in_=h2p_full[:])
                    nc.sync.dma_start(out=dbg['a2f'][:], in_=a2_full[:])

            # ---------------- Phase F: MLP (replicated) -------------------
            if not _inc('F'):
                nc.sync.dma_start(out=t_out[:], in_=chain_sb[:])
            if _inc('F'):
             with (
                tc.tile_pool(name="pf_sb", bufs=1) as fsb,
                tc.tile_pool(name="pf_ps", bufs=1, space="PSUM") as fps,
             ):
                psb = fsb.tile([16, 512], f32, name="psb")
                nc.sync.dma_start(out=psb[:], in_=pool_out[:])
                gt = fsb.tile([16, 512], f32, name="gt")
                nc.vector.tensor_scalar_mul(out=gt[:], in0=psb[:],
                                            scalar1=rc16[:, 0:1])
                fc1c = []
                for c in range(4):
                    fw = fsb.tile([P, 32], f32, name=f"fc1c{c}")
                    nc.sync.dma_start(out=fw[:], in_=t_fc1w[c * P:(c + 1) * P, :])
                    fc1c.append(fw)
                fb1 = fsb.tile([32, 1], f32, name="fb1")
                nc.sync.dma_start(out=fb1[:], in_=t_fc1b[:])
                fw2 = fsb.tile([32, 10], f32, name="fw2")
                nc.sync.dma_start(out=fw2[:], in_=t_fc2w[:])
                fb2 = fsb.tile([16, 10], f32, name="fb2")
                nc.sync.dma_start(out=fb2[:], in_=t_fc2br[:])

                fc1_ps = fps.tile([32, 16], f32, name="fc1_ps")
                for c in range(4):
                    gtt_ps = fps.tile([P, 16], f32, name="gtt_ps", tag="gtt")
                    nc.tensor.transpose(out=gtt_ps[:], in_=gt[:, c * P:(c + 1) * P],
                                        identity=ident[0:16, 0:16])
                    gtt = fsb.tile([P, 16], f32, name="gtt_sb", tag="gtts")
                    nc.vector.tensor_copy(out=gtt[:], in_=gtt_ps[:])
                    nc.tensor.matmul(fc1_ps[:], lhsT=fc1c[c][:],
                                     rhs=gtt[:],
                                     start=(c == 0), stop=(c == 3))
                y1 = fsb.tile([32, 16], f32, name="y1")
                nc.vector.tensor_scalar_add(out=y1[:], in0=fc1_ps[:],
                                            scalar1=fb1[:, 0:1])
                neg1 = fsb.tile([32, 16], f32, name="neg1")
                nc.vector.tensor_scalar_min(out=neg1[:], in0=y1[:], scalar1=0.0)
                en1 = fsb.tile([32, 16], f32, name="en1")
                nc.scalar.activation(en1[:], neg1[:], AF.Exp)
                pm11 = fsb.tile([32, 16], f32, name="pm11")
                nc.vector.tensor_scalar(out=pm11[:], in0=y1[:], scalar1=0.0,
                                        scalar2=-1.0, op0=OP.max, op1=OP.add)
                g2 = fsb.tile([32, 16], f32, name="g2")
                nc.vector.tensor_add(out=g2[:], in0=pm11[:], in1=en1[:])

                fc2_ps = fps.tile([16, 10], f32, name="fc2_ps")
                nc.tensor.matmul(fc2_ps[:], lhsT=g2[:], rhs=fw2[:],
                                 start=True, stop=True)
                osb = fsb.tile([16, 10], f32, name="osb")
                nc.vector.tensor_add(out=osb[:], in0=fc2_ps[:], in1=fb2[:])
                nc.sync.dma_start(out=t_out[:], in_=osb[:])

    nc.compile()
    return nc


def kernel(x, edge_index, batch, W1, att_src1, att_dst1, b1,
           W2, att_src2, att_dst2, b2, fc1_w, fc1_b, fc2_w, fc2_b,
           _trace=False):
    from concourse.bass_utils import run_bass_kernel_spmd
    if _trace:
        try:
            import profile_util
            profile_util.install()
        except Exception:
            pass

    x = np.asarray(x, np.float32)
    W1 = np.asarray(W1, np.float32)
    W2 = np.asarray(W2, np.float32)
    a_s1 = np.asarray(att_src1, np.float32)
    a_d1 = np.asarray(att_dst1, np.float32)
    a_s2 = np.asarray(att_src2, np.float32)
    a_d2 = np.asarray(att_dst2, np.float32)

    pp = _preprocess(np.asarray(edge_index), np.asarray(batch))
    bt = pp['bt']

    if bt not in _PROGRAM_CACHE:
        _PROGRAM_CACHE[bt] = _build_program(bt)
    nc = _PROGRAM_CACHE[bt]

    x_pad = np.zeros((NPAD, P), np.float32)
    x_pad[:N] = x
    V1 = np.zeros((P, 16), np.float32)
    V2 = np.zeros((1024, 16), np.float32)
    for h in range(8):
        V1[:, h] = W1[:, h * P:(h + 1) * P] @ a_s1[h]
        V1[:, 8 + h] = W1[:, h * P:(h + 1) * P] @ a_d1[h]
        V2[:, h] = W2[:, h * 64:(h + 1) * 64] @ a_s2[h]
        V2[:, 8 + h] = W2[:, h * 64:(h + 1) * 64] @ a_d2[h]

    common = {
        "x_pad": x_pad,
        "W1": W1,
        "V1": V1,
        "W2": W2,
        "V2": V2,
        "b1_rep": np.tile(np.asarray(b1, np.float32)[None, :], (P, 1)),
        "b2_rep": np.tile(np.asarray(b2, np.float32)[None, :], (P, 1)),
        "iota128": np.tile(np.arange(P, dtype=np.float32)[None, :], (P, 1)),
        "iota16": np.tile(np.arange(16, dtype=np.float32)[None, :], (P, 1)),
        "recip_cnt16": pp['recip_cnt16'],
        "fc1_w": np.asarray(fc1_w, np.float32),
        "fc1_b": np.asarray(fc1_b, np.float32).reshape(32, 1),
        "fc2_w": np.asarray(fc2_w, np.float32),
        "fc2_b_rep": np.tile(np.asarray(fc2_b, np.float32)[None, :], (16, 1)),
    }
    in_maps = []
    for c in range(NCORES):
        m = dict(common)
        m["src_m"] = pp['src_m'][c]
        m["dst_m"] = pp['dst_m'][c]
        m["seg_m"] = pp['seg_m'][c]
        m["srcl2_m"] = pp['srcl2_m'][c]
        m["dstl2_m"] = pp['dstl2_m'][c]
        m["gid_m"] = pp['gid'][c]
        m["chain"] = np.zeros((16, 10), np.float32)
        in_maps.append(m)

    res = run_bass_kernel_spmd(nc, in_maps, list(range(NCORES)),
                               trace=bool(_trace))
    LAST_PROFILE.clear()
    LAST_PROFILE['exec_time_ns'] = res.exec_time_ns
    LAST_PROFILE['results'] = res
    return np.asarray(res.results[0]["out"], np.float32)



# revision 9
# speedup vs baseline: 1.2869x; 1.2869x over previous
"""GAT (2x GATConv + global_mean_pool + MLP) on 8 Trainium2 NeuronCores.

v2 design (vs baseline):
  - All matmul/elementwise data in bf16 (PE 1 cyc/row vs fp32's 4).
  - Per-tile dma_gather (InstDMAGatherAnt, one launch per table per tile)
    replaces per-block indirect DMAs (~1us SWDGE overhead each).
  - L1 aggregation in transposed orientation: A_hT[c,s] accumulates via
    lhsT = gathered x block, rhs = per-head scaled one-hots; no transpose
    round before the W1 matmuls.
  - dst-side attention logits gathered from small local padded tables.
  - elu via Exp + 2 fused DVE ops; leaky-relu via one scalar_tensor_tensor.
  - AllGather of the [1280,576] bf16 h2p|a2src table split in halves to
    overlap with L1 tile compute.
Node->slot assignment is load-balanced on the host (LPT over 80 dst
tiles) so every (core,tile) has <= bt*128 edges with bt minimal.
"""
import os
import sys
import numpy as np

for _p in ("/opt/trn_rl_repo",):
    if os.path.isdir(_p) and _p not in sys.path:
        sys.path.insert(0, _p)

import ml_dtypes

N = 10000
B = 16
NCORES = 8
P = 128
TPC = 10                    # dst tiles per core
SPC = TPC * P               # slots per core (1280)
NSLOT = NCORES * SPC        # 10240
HALF = SPC // 2             # 640 rows per AG half
NEG = 0.2
XW = 256                    # xs table row (x:128 | asrc:8 | pad) bf16 -> 512B
DW = 128                    # dst-table padded row bf16 -> 256B
HW = 640                    # h2p table row (h2p:512 | a2src:8 | pad) -> 1280B

_PROGRAM_CACHE = {}
LAST_PROFILE = {}

bf = ml_dtypes.bfloat16


def _preprocess(edge_index, batch):
    src = np.concatenate([np.asarray(edge_index[0]), np.arange(N)]).astype(np.int64)
    dst = np.concatenate([np.asarray(edge_index[1]), np.arange(N)]).astype(np.int64)
    batch = np.asarray(batch).astype(np.int64)

    deg = np.bincount(dst, minlength=N)

    # LPT: nodes (by in-degree desc) -> 80 bins of <=128 nodes, then bins ->
    # cores (10 bins each) balancing core edge totals.
    import heapq
    order = np.argsort(-deg, kind='stable')
    nbins = NCORES * TPC
    heap = [(0, b) for b in range(nbins)]
    heapq.heapify(heap)
    bin_nodes = [[] for _ in range(nbins)]
    bin_load = np.zeros(nbins, np.int64)
    spill = []
    for n in order:
        load, b = heapq.heappop(heap)
        bin_nodes[b].append(n)
        bin_load[b] += deg[n]
        if len(bin_nodes[b]) < P:
            heapq.heappush(heap, (bin_load[b], b))
        else:
            spill.append(b)
    # bins -> cores: LPT on bin loads into 8 groups of exactly TPC bins
    border = np.argsort(-bin_load, kind='stable')
    cheap = [(0, TPC, c) for c in range(NCORES)]
    core_bins = [[] for _ in range(NCORES)]
    cload = np.zeros(NCORES, np.int64)
    ccap = [TPC] * NCORES
    import heapq as hq
    ch = [(0, c) for c in range(NCORES)]
    hq.heapify(ch)
    for b in border:
        while True:
            load, c = hq.heappop(ch)
            if ccap[c] > 0:
                break
        core_bins[c].append(b)
        cload[c] += bin_load[b]
        ccap[c] -= 1
        if ccap[c] > 0:
            hq.heappush(ch, (cload[c], c))

    # slot assignment: core c, tile t, slot index within tile by bin order
    slot_of = np.full(N, -1, np.int64)      # global slot 0..NSLOT-1
    for c in range(NCORES):
        for t, b in enumerate(core_bins[c]):
            base = c * SPC + t * P
            for i, n in enumerate(bin_nodes[b]):
                slot_of[n] = base + i
    assert (slot_of >= 0).all()

    owner = slot_of // SPC
    local = slot_of % SPC                     # 0..1279 within owner core
    # std layout row (xs_tab, phase-A order)
    l2std = slot_of
    # h2p_full layout row (single AllGather): owner*1280 + local
    l2ag = slot_of

    # per-edge, grouped by dst (core,tile)
    dslot = slot_of[dst]
    dcore = dslot // SPC
    dtile = (dslot % SPC) // P
    dseg = (dslot % P).astype(np.float32)

    counts = np.zeros((NCORES, TPC), np.int64)
    np.add.at(counts, (dcore, dtile), 1)
    bt = max(int(np.ceil(counts.max() / P)), 1)
    NI = bt * P

    eorder = np.lexsort((src, dtile, dcore))
    src_s, dseg_s = src[eorder], dseg[eorder]
    dcore_s, dtile_s = dcore[eorder], dtile[eorder]
    dloc_s = (slot_of[dst] % SPC)[eorder]

    grp = dcore_s * TPC + dtile_s
    gstart = np.searchsorted(grp, np.arange(NCORES * TPC), 'left')
    rank = np.arange(len(grp)) - gstart[grp]

    # flat edge position k = rank; (p, b) = (k % 128, k // 128)
    srcl1 = np.zeros((NCORES, TPC, NI), np.int16)
    srcl2 = np.zeros((NCORES, TPC, NI), np.int16)
    dloc = np.zeros((NCORES, TPC, NI), np.int16)
    seg = np.full((NCORES, TPC, P, bt), -1.0, np.float32)

    ci = dcore_s
    ti = dtile_s
    srcl1[ci, ti, rank] = l2std[src_s].astype(np.int16)
    srcl2[ci, ti, rank] = l2ag[src_s].astype(np.int16)
    dloc[ci, ti, rank] = dloc_s.astype(np.int16)
    seg[ci, ti, rank % P, rank // P] = dseg_s

    def wrap16(a):                      # [.., NI] -> [.., 128, NI//16]
        w = a.reshape(*a.shape[:-1], NI // 16, 16)
        w = np.swapaxes(w, -1, -2)      # [.., 16, NI//16]
        return np.tile(w, (1, 1, 8, 1)).reshape(*a.shape[:-1], P, NI // 16)

    srcl1_w = wrap16(srcl1)
    srcl2_w = wrap16(srcl2)
    dloc_w = wrap16(dloc)

    gid = np.full((NCORES, TPC, P), -1.0, np.float32)
    for c in range(NCORES):
        for t in range(TPC):
            b = core_bins[c][t]
            for i, n in enumerate(bin_nodes[b]):
                gid[c, t, i] = batch[n]

    cnt = np.zeros(B, np.float32)
    np.add.at(cnt, batch, 1.0)
    recip_cnt16 = (1.0 / np.maximum(cnt, 1.0)).astype(np.float32).reshape(16, 1)

    return dict(bt=bt, srcl1=srcl1_w, srcl2=srcl2_w, dloc=dloc_w, seg=seg,
                gid=gid, recip_cnt16=recip_cnt16, slot_of=slot_of)


def _build_program(bt):
    import concourse.bacc as bacc
    import concourse.bass as bass
    import concourse.mybir as mybir
    import concourse.tile as tile
    from concourse.masks import make_identity
    from concourse import library_config

    f32 = mybir.dt.float32
    bf16 = mybir.dt.bfloat16
    i16 = mybir.dt.int16
    AF = mybir.ActivationFunctionType
    OP = mybir.AluOpType
    IOA = bass.IndirectOffsetOnAxis

    NI = bt * P
    NIW = NI // 16

    nc = bacc.Bacc("TRN2", target_bir_lowering=False, debug=False,
                   enable_asserts=False, num_devices=NCORES)

    # ---------------- inputs ----------------
    t_xs = nc.dram_tensor("xs_host", [NSLOT, XW], bf16, kind="ExternalInput")
    t_xT = nc.dram_tensor("xT_tab", [P, NSLOT], bf16, kind="ExternalInput")
    t_xTloc = nc.dram_tensor("xT_loc", [P, SPC], bf16, kind="ExternalInput")
    t_W1 = nc.dram_tensor("W1b", [P, 1024], bf16, kind="ExternalInput")
    t_V1 = nc.dram_tensor("V1b", [P, 16], bf16, kind="ExternalInput")
    t_W2V2 = nc.dram_tensor("W2V2b", [1024, 528], bf16, kind="ExternalInput")
    t_b1r = nc.dram_tensor("b1_rep", [P, 1024], bf16, kind="ExternalInput")
    t_b2r = nc.dram_tensor("b2_rep", [P, 512], bf16, kind="ExternalInput")
    t_iota = nc.dram_tensor("iota128", [P, P], bf16, kind="ExternalInput")
    t_iota16 = nc.dram_tensor("iota16", [P, 16], bf16, kind="ExternalInput")
    t_rc16 = nc.dram_tensor("recip_cnt16", [16, 1], f32, kind="ExternalInput")
    t_fc1w = nc.dram_tensor("fc1_w", [512, 32], f32, kind="ExternalInput")
    t_fc1b = nc.dram_tensor("fc1_b", [32, 1], f32, kind="ExternalInput")
    t_fc2w = nc.dram_tensor("fc2_w", [32, 10], f32, kind="ExternalInput")
    t_fc2br = nc.dram_tensor("fc2_b_rep", [16, 10], f32, kind="ExternalInput")
    t_srcl1 = nc.dram_tensor("srcl1", [TPC, P, NIW], i16, kind="ExternalInput")
    t_srcl2 = nc.dram_tensor("srcl2", [TPC, P, NIW], i16, kind="ExternalInput")
    t_dloc = nc.dram_tensor("dloc", [TPC, P, NIW], i16, kind="ExternalInput")
    t_seg = nc.dram_tensor("seg_m", [TPC, P, bt], f32, kind="ExternalInput")
    t_gid = nc.dram_tensor("gid_m", [TPC, P], f32, kind="ExternalInput")

    t_out = nc.dram_tensor("out", [16, 10], f32, kind="ExternalOutput")

    with tile.TileContext(nc) as tc:
        with (
            tc.tile_pool(name="const", bufs=1) as csb,
            tc.tile_pool(name="dram", bufs=1, space="DRAM") as dr,
        ):
            # DRAM staging
            xs_tab = dr.tile([NSLOT, XW], bf16)
            a1d_loc = dr.tile([SPC, DW], bf16)
            a2d_loc = dr.tile([SPC, DW], bf16)
            h2p_loc = dr.tile([SPC, HW], bf16)
            h2p_full = dr.tile([NSLOT, HW], bf16, addr_space="Shared")
            pool_in = dr.tile([16, 512], f32)
            pool_out = dr.tile([16, 512], f32, addr_space="Shared")

            nc.gpsimd.load_library(library_config.mlp)

            identb = csb.tile([P, P], bf16)
            make_identity(nc, identb[:])
            iota = csb.tile([P, P], bf16)
            nc.sync.dma_start(out=iota[:], in_=t_iota[:])
            iota16 = csb.tile([P, 16], bf16)
            nc.sync.dma_start(out=iota16[:], in_=t_iota16[:])
            W1sb = csb.tile([P, 1024], bf16)
            nc.sync.dma_start(out=W1sb[:], in_=t_W1[:])
            V1sb = csb.tile([P, 16], bf16)
            nc.sync.dma_start(out=V1sb[:], in_=t_V1[:])
            W2V2sb = []
            for c in range(8):
                w2c = csb.tile([P, 528], bf16, name=f"w2v2c{c}")
                nc.sync.dma_start(out=w2c[:], in_=t_W2V2[c * P:(c + 1) * P, :])
                W2V2sb.append(w2c)
            b1r = csb.tile([P, 1024], bf16)
            nc.sync.dma_start(out=b1r[:], in_=t_b1r[:])
            b2r = csb.tile([P, 512], bf16)
            nc.sync.dma_start(out=b2r[:], in_=t_b2r[:])
            rc16 = csb.tile([16, 1], f32)
            nc.sync.dma_start(out=rc16[:], in_=t_rc16[:])

            # xs staging copy (x cols; alpha cols written by Phase A)
            nc.sync.dma_start(out=xs_tab[:], in_=t_xs[:])

            # ---------------- Phase A: alpha tables ----------------------
            with (
                tc.tile_pool(name="pa_sb", bufs=4) as asb,
                tc.tile_pool(name="pa_ps", bufs=4, space="PSUM") as aps,
            ):
                for g in range(NCORES * TPC):
                    sl = slice(g * P, (g + 1) * P)
                    xt = asb.tile([P, P], bf16, name="xt")
                    nc.sync.dma_start(out=xt[:], in_=t_xT[:, sl])
                    al_ps = aps.tile([P, 16], f32, name="al_ps")
                    nc.tensor.matmul(al_ps[:], lhsT=xt[:], rhs=V1sb[:],
                                     start=True, stop=True)
                    al = asb.tile([P, 16], bf16, name="al")
                    if g % 2 == 0:
                        nc.vector.tensor_copy(out=al[:], in_=al_ps[:])
                    else:
                        nc.scalar.activation(al[:], al_ps[:], AF.Copy)
                    nc.sync.dma_start(out=xs_tab[sl, P:P + 8], in_=al[:, 0:8])
                for t in range(TPC):
                    sl = slice(t * P, (t + 1) * P)
                    xt = asb.tile([P, P], bf16, name="xt2")
                    nc.sync.dma_start(out=xt[:], in_=t_xTloc[:, sl])
                    al_ps = aps.tile([P, 16], f32, name="al_ps2")
                    nc.tensor.matmul(al_ps[:], lhsT=xt[:], rhs=V1sb[:],
                                     start=True, stop=True)
                    al = asb.tile([P, 16], bf16, name="al2")
                    nc.vector.tensor_copy(out=al[:], in_=al_ps[:])
                    nc.sync.dma_start(out=a1d_loc[sl, 0:8], in_=al[:, 8:16])

            # ---------------- L1: aggregation + finalize ------------------
            with (
                tc.tile_pool(name="l1_idx", bufs=3) as isb,
                tc.tile_pool(name="l1_g", bufs=2) as gsb,
                tc.tile_pool(name="l1_sb", bufs=3) as lsb,
                tc.tile_pool(name="l1_fin", bufs=2) as fsb,
                tc.tile_pool(name="l1_ps", bufs=1, space="PSUM") as lps,
            ):
                for t in range(TPC):
                    i1 = isb.tile([P, NIW], i16, name="i1")
                    nc.sync.dma_start(out=i1[:], in_=t_srcl1[t])
                    i2 = isb.tile([P, NIW], i16, name="i2")
                    nc.sync.dma_start(out=i2[:], in_=t_dloc[t])
                    segm = isb.tile([P, bt], f32, name="segm")
                    nc.sync.dma_start(out=segm[:], in_=t_seg[t])

                    xe = gsb.tile([P, bt, XW], bf16, name="xe")
                    nc.gpsimd.dma_gather(
                        out_ap=xe[:], in_ap=xs_tab[:], idxs_ap=i1[:],
                        num_idxs=NI, num_idxs_reg=NI, elem_size=XW,
                        single_packet=False)
                    ade = gsb.tile([P, bt, DW], bf16, name="ade")
                    nc.gpsimd.dma_gather(
                        out_ap=ade[:], in_ap=a1d_loc[:], idxs_ap=i2[:],
                        num_idxs=NI, num_idxs_reg=NI, elem_size=DW,
                        single_packet=False)

                    esum = lsb.tile([P, bt * 8], bf16, name="esum")
                    nc.vector.tensor_tensor(
                        out=esum[:].rearrange("p (b k) -> p b k", b=bt),
                        in0=xe[:, :, P:P + 8], in1=ade[:, :, 0:8], op=OP.add)
                    lrt = lsb.tile([P, bt * 8], bf16, name="lrt")
                    nc.vector.scalar_tensor_tensor(
                        out=lrt[:], in0=esum[:], scalar=NEG, in1=esum[:],
                        op0=OP.mult, op1=OP.max)
                    ex = lsb.tile([P, bt * 8], f32, name="ex")
                    nc.scalar.activation(ex[:], lrt[:], AF.Exp)
                    exb = lsb.tile([P, bt * 8], bf16, name="exb")
                    nc.vector.tensor_copy(out=exb[:], in_=ex[:])

                    agg_ps = lps.tile([P, 1536], f32, name="agg_ps", bufs=1)
                    at_ps = agg_ps[:, 0:1024]
                    den_ps = agg_ps[:, 1024:1032]
                    for b in range(bt):
                        ohx = lsb.tile([P, 1024], bf16, name="ohx", tag="ohx",
                                       bufs=3)
                        for h in range(8):
                            nc.vector.tensor_scalar(
                                out=ohx[:, h * P:(h + 1) * P], in0=iota[:],
                                scalar1=segm[:, b:b + 1],
                                scalar2=ex[:, b * 8 + h:b * 8 + h + 1],
                                op0=OP.is_equal, op1=OP.mult)
                        oh = lsb.tile([P, P], bf16, name="oh", tag="oh", bufs=3)
                        nc.vector.tensor_scalar(
                            out=oh[:], in0=iota[:], scalar1=segm[:, b:b + 1],
                            scalar2=None, op0=OP.is_equal)
                        st = (b == 0)
                        sp = (b == bt - 1)
                        xb = xe[:, b, 0:P]
                        nc.tensor.matmul(at_ps[:, 0:512], lhsT=xb,
                                         rhs=ohx[:, 0:512], start=st, stop=sp)
                        nc.tensor.matmul(at_ps[:, 512:1024], lhsT=xb,
                                         rhs=ohx[:, 512:1024], start=st, stop=sp)
                        nc.tensor.matmul(den_ps, lhsT=oh[:],
                                         rhs=exb[:, b * 8:(b + 1) * 8],
                                         start=st, stop=sp)

                    # ---- finalize tile t
                    den = lsb.tile([P, 8], f32, name="den")
                    nc.vector.tensor_scalar_max(out=den[:], in0=den_ps,
                                                scalar1=1e-30)
                    rec = lsb.tile([P, 8], f32, name="rec")
                    nc.vector.reciprocal(out=rec[:], in_=den[:])

                    y = fsb.tile([P, 1024], bf16, name="y")
                    o1t = None
                    for h in range(8):
                        hs = slice(h * P, (h + 1) * P)
                        at_sb = fsb.tile([P, P], bf16, name="at_sb", tag="ats",
                                         bufs=3)
                        if h % 2 == 0:
                            nc.vector.tensor_copy(out=at_sb[:], in_=at_ps[:, hs])
                        else:
                            nc.scalar.activation(at_sb[:], at_ps[:, hs],
                                                 AF.Copy)
                        if h % 4 == 0:
                            o1t = lps.tile([P, 512], f32, name="o1t",
                                           tag="o1p", bufs=2)
                        o1s = o1t[:, (h % 4) * P:(h % 4 + 1) * P]
                        nc.tensor.matmul(o1s, lhsT=at_sb[:], rhs=W1sb[:, hs],
                                         start=True, stop=True)
                        nc.scalar.activation(y[:, hs], o1s, AF.Copy,
                                             scale=rec[:, h:h + 1])
                    y2 = fsb.tile([P, 1024], bf16, name="y2")
                    nc.vector.tensor_add(out=y2[:], in0=y[:], in1=b1r[:])
                    ee = fsb.tile([P, 1024], bf16, name="ee")
                    nc.scalar.activation(ee[:], y2[:], AF.Exp)
                    u = fsb.tile([P, 1024], bf16, name="u")
                    nc.vector.tensor_scalar(out=u[:], in0=ee[:], scalar1=1.0,
                                            scalar2=-1.0, op0=OP.min, op1=OP.add)
                    e1 = fsb.tile([P, 1024], bf16, name="e1")
                    nc.vector.scalar_tensor_tensor(
                        out=e1[:], in0=y2[:], scalar=0.0, in1=u[:],
                        op0=OP.max, op1=OP.add)

                    h2a2_ps = lps.tile([P, 528], f32, name="h2a2_ps", bufs=1)
                    h2_ps = h2a2_ps[:, 0:512]
                    a2_ps = h2a2_ps[:, 512:528]
                    for h in range(8):
                        hs = slice(h * P, (h + 1) * P)
                        tps = lps.tile([P, P], bf16, name="tps", tag="tps",
                                       bufs=1)
                        nc.tensor.transpose(out=tps[:], in_=e1[:, hs],
                                            identity=identb[:])
                        e1T = fsb.tile([P, P], bf16, name="e1T", tag="e1T",
                                       bufs=3)
                        if h % 2 == 0:
                            nc.vector.tensor_copy(out=e1T[:], in_=tps[:])
                        else:
                            nc.scalar.activation(e1T[:], tps[:], AF.Copy)
                        nc.tensor.matmul(h2_ps, lhsT=e1T[:],
                                         rhs=W2V2sb[h][:, 0:512],
                                         start=(h == 0), stop=(h == 7))
                        nc.tensor.matmul(a2_ps, lhsT=e1T[:],
                                         rhs=W2V2sb[h][:, 512:528],
                                         start=(h == 0), stop=(h == 7))
                    hrow = fsb.tile([P, 520], bf16, name="hrow")
                    nc.vector.tensor_copy(out=hrow[:, 0:512], in_=h2_ps)
                    nc.vector.tensor_copy(out=hrow[:, 512:520],
                                          in_=h2a2_ps[:, 512:520])
                    a2d = lsb.tile([P, 8], bf16, name="a2d")
                    nc.vector.tensor_copy(out=a2d[:], in_=h2a2_ps[:, 520:528])
                    sl = slice(t * P, (t + 1) * P)
                    nc.sync.dma_start(out=h2p_loc[sl, 0:520], in_=hrow[:])
                    nc.sync.dma_start(out=a2d_loc[sl, 0:8], in_=a2d[:])

                nc.gpsimd.collective_compute(
                    "AllGather", mybir.AluOpType.bypass,
                    replica_groups=[list(range(NCORES))],
                    ins=[h2p_loc[:].opt()],
                    outs=[h2p_full[:].opt()])

            # ---------------- L2: aggregation + pool ----------------------
            with (
                tc.tile_pool(name="l2_idx", bufs=3) as isb,
                tc.tile_pool(name="l2_g", bufs=2) as gsb,
                tc.tile_pool(name="l2_sb", bufs=3) as lsb,
                tc.tile_pool(name="l2_fin", bufs=2) as fsb,
                tc.tile_pool(name="l2_ps", bufs=1, space="PSUM") as lps,
            ):
                pool_ps = lps.tile([16, 512], f32, name="pool_ps", bufs=1)
                for t in range(TPC):
                    i1 = isb.tile([P, NIW], i16, name="i1b")
                    nc.sync.dma_start(out=i1[:], in_=t_srcl2[t])
                    i2 = isb.tile([P, NIW], i16, name="i2b")
                    nc.sync.dma_start(out=i2[:], in_=t_dloc[t])
                    segm = isb.tile([P, bt], f32, name="segm2")
                    nc.sync.dma_start(out=segm[:], in_=t_seg[t])

                    hg = gsb.tile([P, bt, HW], bf16, name="hg")
                    nc.gpsimd.dma_gather(
                        out_ap=hg[:], in_ap=h2p_full[:], idxs_ap=i1[:],
                        num_idxs=NI, num_idxs_reg=NI, elem_size=HW,
                        single_packet=False)
                    ade = gsb.tile([P, bt, DW], bf16, name="ade2")
                    nc.gpsimd.dma_gather(
                        out_ap=ade[:], in_ap=a2d_loc[:], idxs_ap=i2[:],
                        num_idxs=NI, num_idxs_reg=NI, elem_size=DW,
                        single_packet=False)

                    esum = lsb.tile([P, bt * 8], bf16, name="esum2")
                    nc.vector.tensor_tensor(
                        out=esum[:].rearrange("p (b k) -> p b k", b=bt),
                        in0=hg[:, :, 512:520], in1=ade[:, :, 0:8], op=OP.add)
                    lrt = lsb.tile([P, bt * 8], bf16, name="lrt2")
                    nc.vector.scalar_tensor_tensor(
                        out=lrt[:], in0=esum[:], scalar=NEG, in1=esum[:],
                        op0=OP.mult, op1=OP.max)
                    ex = lsb.tile([P, bt * 8], f32, name="ex2")
                    nc.scalar.activation(ex[:], lrt[:], AF.Exp)
                    exb = lsb.tile([P, bt * 8], bf16, name="exb2")
                    nc.vector.tensor_copy(out=exb[:], in_=ex[:])

                    haden_ps = lps.tile([P, 528], f32, name="haden_ps", bufs=2)
                    ha_ps = haden_ps[:, 0:512]
                    den_ps = haden_ps[:, 512:520]
                    for b in range(bt):
                        oh = lsb.tile([P, P], bf16, name="oh2", tag="oh2",
                                      bufs=3)
                        nc.vector.tensor_scalar(
                            out=oh[:], in0=iota[:], scalar1=segm[:, b:b + 1],
                            scalar2=None, op0=OP.is_equal)
                        msg = lsb.tile([P, 512], bf16, name="msg", tag="msg",
                                       bufs=3)
                        nc.vector.tensor_tensor(
                            out=msg[:].rearrange("p (h c) -> p h c", h=8),
                            in0=hg[:, b, 0:512].rearrange("p (h c) -> p h c", h=8),
                            in1=exb[:, b * 8:(b + 1) * 8].unsqueeze(2)
                                .broadcast_to([P, 8, 64]),
                            op=OP.mult)
                        st = (b == 0)
                        sp = (b == bt - 1)
                        nc.tensor.matmul(ha_ps, lhsT=oh[:], rhs=msg[:],
                                         start=st, stop=sp)
                        nc.tensor.matmul(den_ps, lhsT=oh[:],
                                         rhs=exb[:, b * 8:(b + 1) * 8],
                                         start=st, stop=sp)

                    # ---- finalize tile t
                    den = lsb.tile([P, 8], f32, name="den2")
                    nc.vector.tensor_scalar_max(out=den[:], in0=den_ps,
                                                scalar1=1e-30)
                    rec = lsb.tile([P, 8], f32, name="rec2")
                    nc.vector.reciprocal(out=rec[:], in_=den[:])
                    ey = fsb.tile([P, 512], bf16, name="ey")
                    for h in range(8):
                        hs = slice(h * 64, (h + 1) * 64)
                        nc.scalar.activation(ey[:, hs], ha_ps[:, hs],
                                             AF.Copy, scale=rec[:, h:h + 1])
                    y2 = fsb.tile([P, 512], bf16, name="y2l2")
                    nc.vector.tensor_add(out=y2[:], in0=ey[:], in1=b2r[:])
                    ee = fsb.tile([P, 512], bf16, name="eel2")
                    nc.scalar.activation(ee[:], y2[:], AF.Exp)
                    u = fsb.tile([P, 512], bf16, name="ul2")
                    nc.vector.tensor_scalar(out=u[:], in0=ee[:], scalar1=1.0,
                                            scalar2=-1.0, op0=OP.min, op1=OP.add)
                    e2 = fsb.tile([P, 512], bf16, name="e2")
                    nc.vector.scalar_tensor_tensor(
                        out=e2[:], in0=y2[:], scalar=0.0, in1=u[:],
                        op0=OP.max, op1=OP.add)

                    gidt = lsb.tile([P, 1], f32, name="gidt")
                    nc.sync.dma_start(out=gidt[:], in_=t_gid[t, :, None])
                    gone = lsb.tile([P, 16], bf16, name="gone")
                    nc.vector.tensor_scalar(out=gone[:], in0=iota16[:],
                                            scalar1=gidt[:, 0:1], scalar2=None,
                                            op0=OP.is_equal)
                    nc.tensor.matmul(pool_ps[:], lhsT=gone[:], rhs=e2[:],
                                     start=(t == 0), stop=(t == TPC - 1))

                pool_sb = lsb.tile([16, 512], f32, name="pool_sb")
                nc.vector.tensor_copy(out=pool_sb[:], in_=pool_ps[:])
                nc.sync.dma_start(out=pool_in[:], in_=pool_sb[:])

            nc.gpsimd.collective_compute(
                "AllReduce", mybir.AluOpType.add,
                replica_groups=[list(range(NCORES))],
                ins=[pool_in[:].opt()], outs=[pool_out[:].opt()])

            # ---------------- MLP (replicated) ----------------------------
            with (
                tc.tile_pool(name="pf_sb", bufs=1) as msb,
                tc.tile_pool(name="pf_ps", bufs=1, space="PSUM") as mps,
            ):
                ident32 = msb.tile([16, 16], mybir.dt.float32, name="id32")
                make_identity(nc, ident32[:])
                psb = msb.tile([16, 512], f32, name="psb")
                nc.sync.dma_start(out=psb[:], in_=pool_out[:])
                gt = msb.tile([16, 512], f32, name="gt")
                nc.vector.tensor_scalar_mul(out=gt[:], in0=psb[:],
                                            scalar1=rc16[:, 0:1])
                fc1c = []
                for c in range(4):
                    fw = msb.tile([P, 32], f32, name=f"fc1c{c}")
                    nc.sync.dma_start(out=fw[:], in_=t_fc1w[c * P:(c + 1) * P, :])
                    fc1c.append(fw)
                fb1 = msb.tile([32, 1], f32, name="fb1")
                nc.sync.dma_start(out=fb1[:], in_=t_fc1b[:])
                fw2 = msb.tile([32, 10], f32, name="fw2")
                nc.sync.dma_start(out=fw2[:], in_=t_fc2w[:])
                fb2 = msb.tile([16, 10], f32, name="fb2")
                nc.sync.dma_start(out=fb2[:], in_=t_fc2br[:])

                fc1_ps = mps.tile([32, 16], f32, name="fc1_ps")
                for c in range(4):
                    gtt_ps = mps.tile([P, 16], f32, name="gtt_ps", tag="gtt")
                    nc.tensor.transpose(out=gtt_ps[:],
                                        in_=gt[:, c * P:(c + 1) * P],
                                        identity=ident32[:])
                    gtt = msb.tile([P, 16], f32, name="gtt_sb", tag="gtts")
                    nc.vector.tensor_copy(out=gtt[:], in_=gtt_ps[:])
                    nc.tensor.matmul(fc1_ps[:], lhsT=fc1c[c][:], rhs=gtt[:],
                                     start=(c == 0), stop=(c == 3))
                y1 = msb.tile([32, 16], f32, name="y1")
                nc.vector.tensor_scalar_add(out=y1[:], in0=fc1_ps[:],
                                            scalar1=fb1[:, 0:1])
                en1 = msb.tile([32, 16], f32, name="en1")
                neg1 = msb.tile([32, 16], f32, name="neg1")
                nc.vector.tensor_scalar_min(out=neg1[:], in0=y1[:], scalar1=0.0)
                nc.scalar.activation(en1[:], neg1[:], AF.Exp)
                pm11 = msb.tile([32, 16], f32, name="pm11")
                nc.vector.tensor_scalar(out=pm11[:], in0=y1[:], scalar1=0.0,
                                        scalar2=-1.0, op0=OP.max, op1=OP.add)
                g2 = msb.tile([32, 16], f32, name="g2")
                nc.vector.tensor_add(out=g2[:], in0=pm11[:], in1=en1[:])

                fc2_ps = mps.tile([16, 10], f32, name="fc2_ps")
                nc.tensor.matmul(fc2_ps[:], lhsT=g2[:], rhs=fw2[:],
                                 start=True, stop=True)
                osb = msb.tile([16, 10], f32, name="osb")
                nc.vector.tensor_add(out=osb[:], in0=fc2_ps[:], in1=fb2[:])
                nc.sync.dma_start(out=t_out[:], in_=osb[:])

    nc.compile()
    return nc


def kernel(x, edge_index, batch, W1, att_src1, att_dst1, b1,
           W2, att_src2, att_dst2, b2, fc1_w, fc1_b, fc2_w, fc2_b,
           _trace=False):
    from concourse.bass_utils import run_bass_kernel_spmd
    if _trace:
        try:
            import profile_util
            profile_util.install()
        except Exception:
            pass

    x = np.asarray(x, np.float32)
    W1 = np.asarray(W1, np.float32)
    W2 = np.asarray(W2, np.float32)
    a_s1 = np.asarray(att_src1, np.float32)
    a_d1 = np.asarray(att_dst1, np.float32)
    a_s2 = np.asarray(att_src2, np.float32)
    a_d2 = np.asarray(att_dst2, np.float32)

    pp = _preprocess(np.asarray(edge_index), np.asarray(batch))
    bt = pp['bt']

    if bt not in _PROGRAM_CACHE:
        _PROGRAM_CACHE[bt] = _build_program(bt)
    nc = _PROGRAM_CACHE[bt]

    V1 = np.zeros((P, 16), np.float32)
    V2 = np.zeros((1024, 16), np.float32)
    for h in range(8):
        V1[:, h] = W1[:, h * P:(h + 1) * P] @ a_s1[h]
        V1[:, 8 + h] = W1[:, h * P:(h + 1) * P] @ a_d1[h]
        V2[:, h] = W2[:, h * 64:(h + 1) * 64] @ a_s2[h]
        V2[:, 8 + h] = W2[:, h * 64:(h + 1) * 64] @ a_d2[h]

    slot_of = pp['slot_of']
    xs_host = np.zeros((NSLOT, XW), bf)
    xs_host[slot_of, 0:P] = x.astype(bf)
    xT = np.zeros((P, NSLOT), bf)
    xT[:, slot_of] = x.T.astype(bf)

    W2V2 = np.concatenate([W2, V2], axis=1).astype(bf)    # [1024, 528]

    common = {
        "xs_host": xs_host,
        "xT_tab": xT,
        "W1b": W1.astype(bf),
        "V1b": V1.astype(bf),
        "W2V2b": W2V2,
        "b1_rep": np.tile(np.asarray(b1, np.float32)[None, :], (P, 1)).astype(bf),
        "b2_rep": np.tile(np.asarray(b2, np.float32)[None, :], (P, 1)).astype(bf),
        "iota128": np.tile(np.arange(P, dtype=np.float32)[None, :], (P, 1)).astype(bf),
        "iota16": np.tile(np.arange(16, dtype=np.float32)[None, :], (P, 1)).astype(bf),
        "recip_cnt16": pp['recip_cnt16'],
        "fc1_w": np.asarray(fc1_w, np.float32),
        "fc1_b": np.asarray(fc1_b, np.float32).reshape(32, 1),
        "fc2_w": np.asarray(fc2_w, np.float32),
        "fc2_b_rep": np.tile(np.asarray(fc2_b, np.float32)[None, :], (16, 1)),
    }
    in_maps = []
    for c in range(NCORES):
        m = dict(common)
        m["xT_loc"] = np.ascontiguousarray(xT[:, c * SPC:(c + 1) * SPC])
        m["srcl1"] = pp['srcl1'][c]
        m["srcl2"] = pp['srcl2'][c]
        m["dloc"] = pp['dloc'][c]
        m["seg_m"] = pp['seg'][c]
        m["gid_m"] = pp['gid'][c]
        in_maps.append(m)

    res = run_bass_kernel_spmd(nc, in_maps, list(range(NCORES)),
                               trace=bool(_trace))
    LAST_PROFILE.clear()
    LAST_PROFILE['exec_time_ns'] = res.exec_time_ns
    LAST_PROFILE['results'] = res
    return np.asarray(res.results[0]["out"], np.float32)


# revision 13
# speedup vs baseline: 1.4193x; 1.1028x over previous
"""GAT (2x GATConv + global_mean_pool + MLP) on 8 Trainium2 NeuronCores.

v2 design (vs baseline):
  - All matmul/elementwise data in bf16 (PE 1 cyc/row vs fp32's 4).
  - Per-tile dma_gather (InstDMAGatherAnt, one launch per table per tile)
    replaces per-block indirect DMAs (~1us SWDGE overhead each).
  - L1 aggregation in transposed orientation: A_hT[c,s] accumulates via
    lhsT = gathered x block, rhs = per-head scaled one-hots; no transpose
    round before the W1 matmuls.
  - dst-side attention logits gathered from small local padded tables.
  - elu via Exp + 2 fused DVE ops; leaky-relu via one scalar_tensor_tensor.
  - AllGather of the [1280,576] bf16 h2p|a2src table split in halves to
    overlap with L1 tile compute.
Node->slot assignment is load-balanced on the host (LPT over 80 dst
tiles) so every (core,tile) has <= bt*128 edges with bt minimal.
"""
import os
import sys
import numpy as np

for _p in ("/opt/trn_rl_repo",):
    if os.path.isdir(_p) and _p not in sys.path:
        sys.path.insert(0, _p)

import ml_dtypes

N = 10000
B = 16
NCORES = 8
P = 128
TPC = 10                    # dst tiles per core
SPC = TPC * P               # slots per core (1280)
NSLOT = NCORES * SPC        # 10240
HALF = SPC // 2             # 640 rows per AG half
NEG = 0.2
XW = 256                    # xs table row (x:128 | asrc:8 | pad) bf16 -> 512B
DW = 128                    # dst-table padded row bf16 -> 256B
HW = 640                    # h2p table row (h2p:512 | a2src:8 | pad) -> 1280B

_PROGRAM_CACHE = {}
LAST_PROFILE = {}

bf = ml_dtypes.bfloat16


def _preprocess(edge_index, batch):
    src = np.concatenate([np.asarray(edge_index[0]), np.arange(N)]).astype(np.int64)
    dst = np.concatenate([np.asarray(edge_index[1]), np.arange(N)]).astype(np.int64)
    batch = np.asarray(batch).astype(np.int64)

    deg = np.bincount(dst, minlength=N)

    # LPT: nodes (by in-degree desc) -> 80 bins of <=128 nodes, then bins ->
    # cores (10 bins each) balancing core edge totals.
    import heapq
    order = np.argsort(-deg, kind='stable')
    nbins = NCORES * TPC
    heap = [(0, b) for b in range(nbins)]
    heapq.heapify(heap)
    bin_nodes = [[] for _ in range(nbins)]
    bin_load = np.zeros(nbins, np.int64)
    spill = []
    for n in order:
        load, b = heapq.heappop(heap)
        bin_nodes[b].append(n)
        bin_load[b] += deg[n]
        if len(bin_nodes[b]) < P:
            heapq.heappush(heap, (bin_load[b], b))
        else:
            spill.append(b)
    # bins -> cores: LPT on bin loads into 8 groups of exactly TPC bins
    border = np.argsort(-bin_load, kind='stable')
    cheap = [(0, TPC, c) for c in range(NCORES)]
    core_bins = [[] for _ in range(NCORES)]
    cload = np.zeros(NCORES, np.int64)
    ccap = [TPC] * NCORES
    import heapq as hq
    ch = [(0, c) for c in range(NCORES)]
    hq.heapify(ch)
    for b in border:
        while True:
            load, c = hq.heappop(ch)
            if ccap[c] > 0:
                break
        core_bins[c].append(b)
        cload[c] += bin_load[b]
        ccap[c] -= 1
        if ccap[c] > 0:
            hq.heappush(ch, (cload[c], c))

    # slot assignment: core c, tile t, slot index within tile by bin order
    slot_of = np.full(N, -1, np.int64)      # global slot 0..NSLOT-1
    for c in range(NCORES):
        for t, b in enumerate(core_bins[c]):
            base = c * SPC + t * P
            for i, n in enumerate(bin_nodes[b]):
                slot_of[n] = base + i
    assert (slot_of >= 0).all()

    owner = slot_of // SPC
    local = slot_of % SPC                     # 0..1279 within owner core
    # std layout row (xs_tab, phase-A order)
    l2std = slot_of
    # h2p_full layout row (single AllGather): owner*1280 + local
    l2ag = slot_of

    # per-edge, grouped by dst (core,tile)
    dslot = slot_of[dst]
    dcore = dslot // SPC
    dtile = (dslot % SPC) // P
    dseg = (dslot % P).astype(np.float32)

    counts = np.zeros((NCORES, TPC), np.int64)
    np.add.at(counts, (dcore, dtile), 1)
    bt = max(int(np.ceil(counts.max() / P)), 1)
    NI = bt * P

    eorder = np.lexsort((src, dtile, dcore))
    src_s, dseg_s = src[eorder], dseg[eorder]
    dcore_s, dtile_s = dcore[eorder], dtile[eorder]
    dloc_s = (slot_of[dst] % SPC)[eorder]

    grp = dcore_s * TPC + dtile_s
    gstart = np.searchsorted(grp, np.arange(NCORES * TPC), 'left')
    rank = np.arange(len(grp)) - gstart[grp]

    # flat edge position k = rank; (p, b) = (k % 128, k // 128)
    srcl1 = np.zeros((NCORES, TPC, NI), np.int16)
    srcl2 = np.zeros((NCORES, TPC, NI), np.int16)
    dloc = np.zeros((NCORES, TPC, NI), np.int16)
    seg = np.full((NCORES, TPC, P, bt), -1.0, np.float32)

    ci = dcore_s
    ti = dtile_s
    srcl1[ci, ti, rank] = l2std[src_s].astype(np.int16)
    srcl2[ci, ti, rank] = l2ag[src_s].astype(np.int16)
    dloc[ci, ti, rank] = dloc_s.astype(np.int16)
    seg[ci, ti, rank % P, rank // P] = dseg_s

    def wrap16(a):                      # [.., NI] -> [.., 128, NI//16]
        w = a.reshape(*a.shape[:-1], NI // 16, 16)
        w = np.swapaxes(w, -1, -2)      # [.., 16, NI//16]
        return np.tile(w, (1, 1, 8, 1)).reshape(*a.shape[:-1], P, NI // 16)

    srcl1_w = wrap16(srcl1)
    srcl2_w = wrap16(srcl2)

    gid = np.full((NCORES, TPC, P), -1.0, np.float32)
    for c in range(NCORES):
        for t in range(TPC):
            b = core_bins[c][t]
            for i, n in enumerate(bin_nodes[b]):
                gid[c, t, i] = batch[n]

    cnt = np.zeros(B, np.float32)
    np.add.at(cnt, batch, 1.0)
    recip_cnt16 = (1.0 / np.maximum(cnt, 1.0)).astype(np.float32).reshape(16, 1)

    return dict(bt=bt, srcl1=srcl1_w, srcl2=srcl2_w, seg=seg,
                gid=gid, recip_cnt16=recip_cnt16, slot_of=slot_of)


def _build_program(bt):
    import concourse.bacc as bacc
    import concourse.bass as bass
    import concourse.mybir as mybir
    import concourse.tile as tile
    from concourse.masks import make_identity
    from concourse import library_config

    f32 = mybir.dt.float32
    bf16 = mybir.dt.bfloat16
    i16 = mybir.dt.int16
    AF = mybir.ActivationFunctionType
    OP = mybir.AluOpType
    IOA = bass.IndirectOffsetOnAxis

    NI = bt * P
    NIW = NI // 16

    nc = bacc.Bacc("TRN2", target_bir_lowering=False, debug=False,
                   enable_asserts=False, num_devices=NCORES)

    # ---------------- inputs ----------------
    t_xs = nc.dram_tensor("xs_host", [NSLOT, XW], bf16, kind="ExternalInput")
    t_xT = nc.dram_tensor("xT_tab", [P, NSLOT], bf16, kind="ExternalInput")
    t_xTloc = nc.dram_tensor("xT_loc", [P, SPC], bf16, kind="ExternalInput")
    t_W1 = nc.dram_tensor("W1b", [P, 1024], bf16, kind="ExternalInput")
    t_V1 = nc.dram_tensor("V1b", [P, 16], bf16, kind="ExternalInput")
    t_W2V2 = nc.dram_tensor("W2V2b", [1024, 528], bf16, kind="ExternalInput")
    t_b1r = nc.dram_tensor("b1_rep", [P, 1024], bf16, kind="ExternalInput")
    t_b2r = nc.dram_tensor("b2_rep", [P, 512], bf16, kind="ExternalInput")
    t_iota = nc.dram_tensor("iota128", [P, P], bf16, kind="ExternalInput")
    t_iota16 = nc.dram_tensor("iota16", [P, 16], bf16, kind="ExternalInput")
    t_rc16 = nc.dram_tensor("recip_cnt16", [16, 1], f32, kind="ExternalInput")
    t_fc1w = nc.dram_tensor("fc1_w", [512, 32], f32, kind="ExternalInput")
    t_fc1b = nc.dram_tensor("fc1_b", [32, 1], f32, kind="ExternalInput")
    t_fc2w = nc.dram_tensor("fc2_w", [32, 10], f32, kind="ExternalInput")
    t_fc2br = nc.dram_tensor("fc2_b_rep", [16, 10], f32, kind="ExternalInput")
    t_srcl1 = nc.dram_tensor("srcl1", [TPC, P, NIW], i16, kind="ExternalInput")
    t_srcl2 = nc.dram_tensor("srcl2", [TPC, P, NIW], i16, kind="ExternalInput")
    t_iota8x = nc.dram_tensor("iota8x", [P, 1024], bf16, kind="ExternalInput")
    t_seg = nc.dram_tensor("seg_m", [TPC, P, bt], f32, kind="ExternalInput")
    t_gid = nc.dram_tensor("gid_m", [TPC, P], f32, kind="ExternalInput")

    t_out = nc.dram_tensor("out", [16, 10], f32, kind="ExternalOutput")

    with tile.TileContext(nc) as tc:
        with (
            tc.tile_pool(name="const", bufs=1) as csb,
            tc.tile_pool(name="dram", bufs=1, space="DRAM") as dr,
        ):
            # DRAM staging
            xs_tab = dr.tile([NSLOT, XW], bf16)
            a1d_loc = dr.tile([SPC, 8], bf16)
            h2p_loc = dr.tile([SPC, HW], bf16)
            h2p_full = dr.tile([NSLOT, HW], bf16)
            h2p_chunks = [dr.tile([NCORES * P, HW], bf16, addr_space="Shared",
                                  name=f"h2p_chunk{_t}") for _t in range(TPC)]
            pool_in = dr.tile([16, 512], f32)
            pool_out = dr.tile([16, 512], f32, addr_space="Shared")

            nc.gpsimd.load_library(library_config.mlp)

            identb = csb.tile([P, P], bf16)
            make_identity(nc, identb[:])
            iota = csb.tile([P, P], bf16)
            nc.sync.dma_start(out=iota[:], in_=t_iota[:])
            iota8x = csb.tile([P, 1024], bf16)
            nc.sync.dma_start(out=iota8x[:], in_=t_iota8x[:])
            a2d_keep = csb.tile([P, 8 * TPC], bf16)
            iota16 = csb.tile([P, 16], bf16)
            nc.sync.dma_start(out=iota16[:], in_=t_iota16[:])
            W1sb = csb.tile([P, 1024], bf16)
            nc.sync.dma_start(out=W1sb[:], in_=t_W1[:])
            V1sb = csb.tile([P, 16], bf16)
            nc.sync.dma_start(out=V1sb[:], in_=t_V1[:])
            W2V2sb = []
            for c in range(8):
                w2c = csb.tile([P, 528], bf16, name=f"w2v2c{c}")
                nc.sync.dma_start(out=w2c[:], in_=t_W2V2[c * P:(c + 1) * P, :])
                W2V2sb.append(w2c)
            b1r = csb.tile([P, 1024], bf16)
            nc.sync.dma_start(out=b1r[:], in_=t_b1r[:])
            b2r = csb.tile([P, 512], bf16)
            nc.sync.dma_start(out=b2r[:], in_=t_b2r[:])
            rc16 = csb.tile([16, 1], f32)
            nc.sync.dma_start(out=rc16[:], in_=t_rc16[:])

            # xs staging copy (x cols; alpha cols written by Phase A)
            nc.sync.dma_start(out=xs_tab[:], in_=t_xs[:])

            # ---------------- Phase A: alpha tables ----------------------
            with (
                tc.tile_pool(name="pa_sb", bufs=4) as asb,
                tc.tile_pool(name="pa_ps", bufs=4, space="PSUM") as aps,
            ):
                for g in range(NCORES * TPC):
                    sl = slice(g * P, (g + 1) * P)
                    xt = asb.tile([P, P], bf16, name="xt")
                    nc.sync.dma_start(out=xt[:], in_=t_xT[:, sl])
                    al_ps = aps.tile([P, 16], f32, name="al_ps")
                    nc.tensor.matmul(al_ps[:], lhsT=xt[:], rhs=V1sb[:],
                                     start=True, stop=True)
                    al = asb.tile([P, 16], bf16, name="al")
                    if g % 2 == 0:
                        nc.vector.tensor_copy(out=al[:], in_=al_ps[:])
                    else:
                        nc.scalar.activation(al[:], al_ps[:], AF.Copy)
                    nc.sync.dma_start(out=xs_tab[sl, P:P + 8], in_=al[:, 0:8])
                for t in range(TPC):
                    sl = slice(t * P, (t + 1) * P)
                    xt = asb.tile([P, P], bf16, name="xt2")
                    nc.sync.dma_start(out=xt[:], in_=t_xTloc[:, sl])
                    al_ps = aps.tile([P, 16], f32, name="al_ps2")
                    nc.tensor.matmul(al_ps[:], lhsT=xt[:], rhs=V1sb[:],
                                     start=True, stop=True)
                    al = asb.tile([P, 16], bf16, name="al2")
                    nc.vector.tensor_copy(out=al[:], in_=al_ps[:])
                    nc.sync.dma_start(out=a1d_loc[sl, :], in_=al[:, 8:16])

            # ---------------- L1: aggregation + finalize ------------------
            with (
                tc.tile_pool(name="l1_idx", bufs=3) as isb,
                tc.tile_pool(name="l1_g", bufs=2) as gsb,
                tc.tile_pool(name="l1_sb", bufs=3) as lsb,
                tc.tile_pool(name="l1_fin", bufs=2) as fsb,
                tc.tile_pool(name="l1_ps", bufs=1, space="PSUM") as lps,
            ):
                for t in range(TPC):
                    i1 = isb.tile([P, NIW], i16, name="i1")
                    nc.sync.dma_start(out=i1[:], in_=t_srcl1[t])
                    segm = isb.tile([P, bt], f32, name="segm")
                    nc.sync.dma_start(out=segm[:], in_=t_seg[t])
                    a1dt = isb.tile([P, 8], bf16, name="a1dt")
                    nc.sync.dma_start(out=a1dt[:],
                                      in_=a1d_loc[t * P:(t + 1) * P, :])

                    xe = gsb.tile([P, bt, XW], bf16, name="xe")
                    nc.gpsimd.dma_gather(
                        out_ap=xe[:], in_ap=xs_tab[:], idxs_ap=i1[:],
                        num_idxs=NI, num_idxs_reg=NI, elem_size=XW,
                        single_packet=False)

                    agg_ps = lps.tile([P, 1536], f32, name="agg_ps", bufs=1)
                    at_ps = agg_ps[:, 0:1024]
                    den_ps = agg_ps[:, 1024:1032]
                    adp = agg_ps[:, 1040:1040 + bt * 8]

                    # pass 1: one-hots + dst-logit expansion via PE
                    ohs = []
                    for b in range(bt):
                        oh = lsb.tile([P, P], bf16, name=f"oh{b}", tag=f"oh{b}",
                                      bufs=2)
                        nc.vector.tensor_scalar(
                            out=oh[:], in0=iota[:], scalar1=segm[:, b:b + 1],
                            scalar2=None, op0=OP.is_equal)
                        ohs.append(oh)
                        tps = lps.tile([P, P], bf16, name="tps1", tag="tps",
                                       bufs=1)
                        nc.tensor.transpose(out=tps[:], in_=oh[:],
                                            identity=identb[:])
                        ohT = lsb.tile([P, P], bf16, name="ohT", tag="ohT",
                                       bufs=3)
                        if b % 2 == 0:
                            nc.vector.tensor_copy(out=ohT[:], in_=tps[:])
                        else:
                            nc.scalar.activation(ohT[:], tps[:], AF.Copy)
                        nc.tensor.matmul(adp[:, b * 8:(b + 1) * 8], lhsT=ohT[:],
                                         rhs=a1dt[:], start=True, stop=True)

                    esum = lsb.tile([P, bt * 8], bf16, name="esum")
                    nc.vector.tensor_tensor(
                        out=esum[:].rearrange("p (b k) -> p b k", b=bt),
                        in0=xe[:, :, P:P + 8],
                        in1=adp.rearrange("p (b k) -> p b k", b=bt), op=OP.add)
                    lrt = lsb.tile([P, bt * 8], bf16, name="lrt")
                    nc.vector.scalar_tensor_tensor(
                        out=lrt[:], in0=esum[:], scalar=NEG, in1=esum[:],
                        op0=OP.mult, op1=OP.max)
                    exb = lsb.tile([P, bt * 8], bf16, name="exb")
                    nc.scalar.activation(exb[:], lrt[:], AF.Exp)

                    # pass 2: scaled one-hots (one wide stt) + aggregation
                    for b in range(bt):
                        ohx = lsb.tile([P, 1024], bf16, name="ohx", tag="ohx",
                                       bufs=3)
                        nc.vector.scalar_tensor_tensor(
                            out=ohx[:].rearrange("p (g s) -> p g s", g=8),
                            in0=iota8x[:].rearrange("p (g s) -> p g s", g=8),
                            scalar=segm[:, b:b + 1],
                            in1=exb[:, b * 8:(b + 1) * 8].unsqueeze(2)
                                .broadcast_to([P, 8, P]),
                            op0=OP.is_equal, op1=OP.mult)
                        st = (b == 0)
                        sp = (b == bt - 1)
                        xb = xe[:, b, 0:P]
                        nc.tensor.matmul(at_ps[:, 0:512], lhsT=xb,
                                         rhs=ohx[:, 0:512], start=st, stop=sp)
                        nc.tensor.matmul(at_ps[:, 512:1024], lhsT=xb,
                                         rhs=ohx[:, 512:1024], start=st, stop=sp)
                        nc.tensor.matmul(den_ps, lhsT=ohs[b][:],
                                         rhs=exb[:, b * 8:(b + 1) * 8],
                                         start=st, stop=sp)

                    # ---- finalize tile t
                    den = lsb.tile([P, 8], f32, name="den")
                    nc.vector.tensor_scalar_max(out=den[:], in0=den_ps,
                                                scalar1=1e-30)
                    rec = lsb.tile([P, 8], f32, name="rec")
                    nc.vector.reciprocal(out=rec[:], in_=den[:])

                    y = fsb.tile([P, 1024], bf16, name="y")
                    o1t = None
                    for h in range(8):
                        hs = slice(h * P, (h + 1) * P)
                        at_sb = fsb.tile([P, P], bf16, name="at_sb", tag="ats",
                                         bufs=3)
                        if h % 2 == 0:
                            nc.vector.tensor_copy(out=at_sb[:], in_=at_ps[:, hs])
                        else:
                            nc.scalar.activation(at_sb[:], at_ps[:, hs],
                                                 AF.Copy)
                        if h % 4 == 0:
                            o1t = lps.tile([P, 512], f32, name="o1t",
                                           tag="o1p", bufs=2)
                        o1s = o1t[:, (h % 4) * P:(h % 4 + 1) * P]
                        nc.tensor.matmul(o1s, lhsT=at_sb[:], rhs=W1sb[:, hs],
                                         start=True, stop=True)
                        nc.scalar.activation(y[:, hs], o1s, AF.Copy,
                                             scale=rec[:, h:h + 1])
                    y2 = fsb.tile([P, 1024], bf16, name="y2")
                    nc.vector.tensor_add(out=y2[:], in0=y[:], in1=b1r[:])
                    ee = fsb.tile([P, 1024], bf16, name="ee")
                    nc.scalar.activation(ee[:], y2[:], AF.Exp)
                    u = fsb.tile([P, 1024], bf16, name="u")
                    nc.vector.tensor_scalar(out=u[:], in0=ee[:], scalar1=1.0,
                                            scalar2=-1.0, op0=OP.min, op1=OP.add)
                    e1 = fsb.tile([P, 1024], bf16, name="e1")
                    nc.vector.scalar_tensor_tensor(
                        out=e1[:], in0=y2[:], scalar=0.0, in1=u[:],
                        op0=OP.max, op1=OP.add)

                    h2a2_ps = lps.tile([P, 528], f32, name="h2a2_ps", bufs=1)
                    h2_ps = h2a2_ps[:, 0:512]
                    a2_ps = h2a2_ps[:, 512:528]
                    for h in range(8):
                        hs = slice(h * P, (h + 1) * P)
                        tps = lps.tile([P, P], bf16, name="tps", tag="tps",
                                       bufs=1)
                        nc.tensor.transpose(out=tps[:], in_=e1[:, hs],
                                            identity=identb[:])
                        e1T = fsb.tile([P, P], bf16, name="e1T", tag="e1T",
                                       bufs=3)
                        if h % 2 == 0:
                            nc.vector.tensor_copy(out=e1T[:], in_=tps[:])
                        else:
                            nc.scalar.activation(e1T[:], tps[:], AF.Copy)
                        nc.tensor.matmul(h2_ps, lhsT=e1T[:],
                                         rhs=W2V2sb[h][:, 0:512],
                                         start=(h == 0), stop=(h == 7))
                        nc.tensor.matmul(a2_ps, lhsT=e1T[:],
                                         rhs=W2V2sb[h][:, 512:528],
                                         start=(h == 0), stop=(h == 7))
                    hrow = fsb.tile([P, 520], bf16, name="hrow")
                    nc.vector.tensor_copy(out=hrow[:, 0:512], in_=h2_ps)
                    nc.vector.tensor_copy(out=hrow[:, 512:520],
                                          in_=h2a2_ps[:, 512:520])
                    nc.vector.tensor_copy(out=a2d_keep[:, t * 8:(t + 1) * 8],
                                          in_=h2a2_ps[:, 520:528])
                    sl = slice(t * P, (t + 1) * P)
                    nc.sync.dma_start(out=h2p_loc[sl, 0:520], in_=hrow[:])

                    nc.gpsimd.collective_compute(
                        "AllGather", mybir.AluOpType.bypass,
                        replica_groups=[list(range(NCORES))],
                        ins=[h2p_loc[sl, :].opt()],
                        outs=[h2p_chunks[t][:].opt()])
                    full_v = h2p_full[:].rearrange(
                        "(c tt p) w -> c tt p w", c=NCORES, tt=TPC)[:, t]
                    chunk_v = h2p_chunks[t][:].rearrange(
                        "(c p) w -> c p w", c=NCORES)
                    nc.sync.dma_start(out=full_v, in_=chunk_v)

            # ---------------- L2: aggregation + pool ----------------------
            with (
                tc.tile_pool(name="l2_idx", bufs=3) as isb,
                tc.tile_pool(name="l2_g", bufs=2) as gsb,
                tc.tile_pool(name="l2_sb", bufs=3) as lsb,
                tc.tile_pool(name="l2_fin", bufs=2) as fsb,
                tc.tile_pool(name="l2_ps", bufs=1, space="PSUM") as lps,
            ):
                pool_ps = lps.tile([16, 512], f32, name="pool_ps", bufs=1)
                for t in range(TPC):
                    i1 = isb.tile([P, NIW], i16, name="i1b")
                    nc.sync.dma_start(out=i1[:], in_=t_srcl2[t])
                    segm = isb.tile([P, bt], f32, name="segm2")
                    nc.sync.dma_start(out=segm[:], in_=t_seg[t])

                    hg = gsb.tile([P, bt, HW], bf16, name="hg")
                    nc.gpsimd.dma_gather(
                        out_ap=hg[:], in_ap=h2p_full[:], idxs_ap=i1[:],
                        num_idxs=NI, num_idxs_reg=NI, elem_size=HW,
                        single_packet=False)

                    haden_ps = lps.tile([P, 768], f32, name="haden_ps", bufs=2)
                    ha_ps = haden_ps[:, 0:512]
                    den_ps = haden_ps[:, 512:520]
                    adp = haden_ps[:, 528:528 + bt * 8]

                    ohs = []
                    for b in range(bt):
                        oh = lsb.tile([P, P], bf16, name=f"oh2_{b}",
                                      tag=f"oh2_{b}", bufs=2)
                        nc.vector.tensor_scalar(
                            out=oh[:], in0=iota[:], scalar1=segm[:, b:b + 1],
                            scalar2=None, op0=OP.is_equal)
                        ohs.append(oh)
                        tps = lps.tile([P, P], bf16, name="tps2", tag="tps2",
                                       bufs=1)
                        nc.tensor.transpose(out=tps[:], in_=oh[:],
                                            identity=identb[:])
                        ohT = lsb.tile([P, P], bf16, name="ohT2", tag="ohT2",
                                       bufs=3)
                        if b % 2 == 0:
                            nc.vector.tensor_copy(out=ohT[:], in_=tps[:])
                        else:
                            nc.scalar.activation(ohT[:], tps[:], AF.Copy)
                        nc.tensor.matmul(adp[:, b * 8:(b + 1) * 8], lhsT=ohT[:],
                                         rhs=a2d_keep[:, t * 8:(t + 1) * 8],
                                         start=True, stop=True)

                    esum = lsb.tile([P, bt * 8], bf16, name="esum2")
                    nc.vector.tensor_tensor(
                        out=esum[:].rearrange("p (b k) -> p b k", b=bt),
                        in0=hg[:, :, 512:520],
                        in1=adp.rearrange("p (b k) -> p b k", b=bt), op=OP.add)
                    lrt = lsb.tile([P, bt * 8], bf16, name="lrt2")
                    nc.vector.scalar_tensor_tensor(
                        out=lrt[:], in0=esum[:], scalar=NEG, in1=esum[:],
                        op0=OP.mult, op1=OP.max)
                    exb = lsb.tile([P, bt * 8], bf16, name="exb2")
                    nc.scalar.activation(exb[:], lrt[:], AF.Exp)

                    for b in range(bt):
                        msg = lsb.tile([P, 512], bf16, name="msg", tag="msg",
                                       bufs=3)
                        nc.vector.tensor_tensor(
                            out=msg[:].rearrange("p (h c) -> p h c", h=8),
                            in0=hg[:, b, 0:512].rearrange("p (h c) -> p h c", h=8),
                            in1=exb[:, b * 8:(b + 1) * 8].unsqueeze(2)
                                .broadcast_to([P, 8, 64]),
                            op=OP.mult)
                        st = (b == 0)
                        sp = (b == bt - 1)
                        nc.tensor.matmul(ha_ps, lhsT=ohs[b][:], rhs=msg[:],
                                         start=st, stop=sp)
                        nc.tensor.matmul(den_ps, lhsT=ohs[b][:],
                                         rhs=exb[:, b * 8:(b + 1) * 8],
                                         start=st, stop=sp)

                    # ---- finalize tile t
                    den = lsb.tile([P, 8], f32, name="den2")
                    nc.vector.tensor_scalar_max(out=den[:], in0=den_ps,
                                                scalar1=1e-30)
                    rec = lsb.tile([P, 8], f32, name="rec2")
                    nc.vector.reciprocal(out=rec[:], in_=den[:])
                    ey = fsb.tile([P, 512], bf16, name="ey")
                    for h in range(8):
                        hs = slice(h * 64, (h + 1) * 64)
                        nc.scalar.activation(ey[:, hs], ha_ps[:, hs],
                                             AF.Copy, scale=rec[:, h:h + 1])
                    y2 = fsb.tile([P, 512], bf16, name="y2l2")
                    nc.vector.tensor_add(out=y2[:], in0=ey[:], in1=b2r[:])
                    ee = fsb.tile([P, 512], bf16, name="eel2")
                    nc.scalar.activation(ee[:], y2[:], AF.Exp)
                    u = fsb.tile([P, 512], bf16, name="ul2")
                    nc.vector.tensor_scalar(out=u[:], in0=ee[:], scalar1=1.0,
                                            scalar2=-1.0, op0=OP.min, op1=OP.add)
                    e2 = fsb.tile([P, 512], bf16, name="e2")
                    nc.vector.scalar_tensor_tensor(
                        out=e2[:], in0=y2[:], scalar=0.0, in1=u[:],
                        op0=OP.max, op1=OP.add)

                    gidt = lsb.tile([P, 1], f32, name="gidt")
                    nc.sync.dma_start(out=gidt[:], in_=t_gid[t, :, None])
                    gone = lsb.tile([P, 16], bf16, name="gone")
                    nc.vector.tensor_scalar(out=gone[:], in0=iota16[:],
                                            scalar1=gidt[:, 0:1], scalar2=None,
                                            op0=OP.is_equal)
                    nc.tensor.matmul(pool_ps[:], lhsT=gone[:], rhs=e2[:],
                                     start=(t == 0), stop=(t == TPC - 1))

                pool_sb = lsb.tile([16, 512], f32, name="pool_sb")
                nc.vector.tensor_copy(out=pool_sb[:], in_=pool_ps[:])
                nc.sync.dma_start(out=pool_in[:], in_=pool_sb[:])

            nc.gpsimd.collective_compute(
                "AllReduce", mybir.AluOpType.add,
                replica_groups=[list(range(NCORES))],
                ins=[pool_in[:].opt()], outs=[pool_out[:].opt()])

            # ---------------- MLP (replicated) ----------------------------
            with (
                tc.tile_pool(name="pf_sb", bufs=1) as msb,
                tc.tile_pool(name="pf_ps", bufs=1, space="PSUM") as mps,
            ):
                ident32 = msb.tile([16, 16], mybir.dt.float32, name="id32")
                make_identity(nc, ident32[:])
                psb = msb.tile([16, 512], f32, name="psb")
                nc.sync.dma_start(out=psb[:], in_=pool_out[:])
                gt = msb.tile([16, 512], f32, name="gt")
                nc.vector.tensor_scalar_mul(out=gt[:], in0=psb[:],
                                            scalar1=rc16[:, 0:1])
                fc1c = []
                for c in range(4):
                    fw = msb.tile([P, 32], f32, name=f"fc1c{c}")
                    nc.sync.dma_start(out=fw[:], in_=t_fc1w[c * P:(c + 1) * P, :])
                    fc1c.append(fw)
                fb1 = msb.tile([32, 1], f32, name="fb1")
                nc.sync.dma_start(out=fb1[:], in_=t_fc1b[:])
                fw2 = msb.tile([32, 10], f32, name="fw2")
                nc.sync.dma_start(out=fw2[:], in_=t_fc2w[:])
                fb2 = msb.tile([16, 10], f32, name="fb2")
                nc.sync.dma_start(out=fb2[:], in_=t_fc2br[:])

                fc1_ps = mps.tile([32, 16], f32, name="fc1_ps")
                for c in range(4):
                    gtt_ps = mps.tile([P, 16], f32, name="gtt_ps", tag="gtt")
                    nc.tensor.transpose(out=gtt_ps[:],
                                        in_=gt[:, c * P:(c + 1) * P],
                                        identity=ident32[:])
                    gtt = msb.tile([P, 16], f32, name="gtt_sb", tag="gtts")
                    nc.vector.tensor_copy(out=gtt[:], in_=gtt_ps[:])
                    nc.tensor.matmul(fc1_ps[:], lhsT=fc1c[c][:], rhs=gtt[:],
                                     start=(c == 0), stop=(c == 3))
                y1 = msb.tile([32, 16], f32, name="y1")
                nc.vector.tensor_scalar_add(out=y1[:], in0=fc1_ps[:],
                                            scalar1=fb1[:, 0:1])
                en1 = msb.tile([32, 16], f32, name="en1")
                neg1 = msb.tile([32, 16], f32, name="neg1")
                nc.vector.tensor_scalar_min(out=neg1[:], in0=y1[:], scalar1=0.0)
                nc.scalar.activation(en1[:], neg1[:], AF.Exp)
                pm11 = msb.tile([32, 16], f32, name="pm11")
                nc.vector.tensor_scalar(out=pm11[:], in0=y1[:], scalar1=0.0,
                                        scalar2=-1.0, op0=OP.max, op1=OP.add)
                g2 = msb.tile([32, 16], f32, name="g2")
                nc.vector.tensor_add(out=g2[:], in0=pm11[:], in1=en1[:])

                fc2_ps = mps.tile([16, 10], f32, name="fc2_ps")
                nc.tensor.matmul(fc2_ps[:], lhsT=g2[:], rhs=fw2[:],
                                 start=True, stop=True)
                osb = msb.tile([16, 10], f32, name="osb")
                nc.vector.tensor_add(out=osb[:], in0=fc2_ps[:], in1=fb2[:])
                nc.sync.dma_start(out=t_out[:], in_=osb[:])

    nc.compile()
    return nc


def kernel(x, edge_index, batch, W1, att_src1, att_dst1, b1,
           W2, att_src2, att_dst2, b2, fc1_w, fc1_b, fc2_w, fc2_b,
           _trace=False):
    from concourse.bass_utils import run_bass_kernel_spmd
    if _trace:
        try:
            import profile_util
            profile_util.install()
        except Exception:
            pass

    x = np.asarray(x, np.float32)
    W1 = np.asarray(W1, np.float32)
    W2 = np.asarray(W2, np.float32)
    a_s1 = np.asarray(att_src1, np.float32)
    a_d1 = np.asarray(att_dst1, np.float32)
    a_s2 = np.asarray(att_src2, np.float32)
    a_d2 = np.asarray(att_dst2, np.float32)

    pp = _preprocess(np.asarray(edge_index), np.asarray(batch))
    bt = pp['bt']

    if bt not in _PROGRAM_CACHE:
        _PROGRAM_CACHE[bt] = _build_program(bt)
    nc = _PROGRAM_CACHE[bt]

    V1 = np.zeros((P, 16), np.float32)
    V2 = np.zeros((1024, 16), np.float32)
    for h in range(8):
        V1[:, h] = W1[:, h * P:(h + 1) * P] @ a_s1[h]
        V1[:, 8 + h] = W1[:, h * P:(h + 1) * P] @ a_d1[h]
        V2[:, h] = W2[:, h * 64:(h + 1) * 64] @ a_s2[h]
        V2[:, 8 + h] = W2[:, h * 64:(h + 1) * 64] @ a_d2[h]

    slot_of = pp['slot_of']
    xs_host = np.zeros((NSLOT, XW), bf)
    xs_host[slot_of, 0:P] = x.astype(bf)
    xT = np.zeros((P, NSLOT), bf)
    xT[:, slot_of] = x.T.astype(bf)

    W2V2 = np.concatenate([W2, V2], axis=1).astype(bf)    # [1024, 528]

    common = {
        "xs_host": xs_host,
        "xT_tab": xT,
        "W1b": W1.astype(bf),
        "V1b": V1.astype(bf),
        "W2V2b": W2V2,
        "b1_rep": np.tile(np.asarray(b1, np.float32)[None, :], (P, 1)).astype(bf),
        "b2_rep": np.tile(np.asarray(b2, np.float32)[None, :], (P, 1)).astype(bf),
        "iota128": np.tile(np.arange(P, dtype=np.float32)[None, :], (P, 1)).astype(bf),
        "iota8x": np.tile(np.arange(P, dtype=np.float32)[None, :], (P, 8)).astype(bf),
        "iota16": np.tile(np.arange(16, dtype=np.float32)[None, :], (P, 1)).astype(bf),
        "recip_cnt16": pp['recip_cnt16'],
        "fc1_w": np.asarray(fc1_w, np.float32),
        "fc1_b": np.asarray(fc1_b, np.float32).reshape(32, 1),
        "fc2_w": np.asarray(fc2_w, np.float32),
        "fc2_b_rep": np.tile(np.asarray(fc2_b, np.float32)[None, :], (16, 1)),
    }
    in_maps = []
    for c in range(NCORES):
        m = dict(common)
        m["xT_loc"] = np.ascontiguousarray(xT[:, c * SPC:(c + 1) * SPC])
        m["srcl1"] = pp['srcl1'][c]
        m["srcl2"] = pp['srcl2'][c]
        m["seg_m"] = pp['seg'][c]
        m["gid_m"] = pp['gid'][c]
        in_maps.append(m)

    res = run_bass_kernel_spmd(nc, in_maps, list(range(NCORES)),
                               trace=bool(_trace))
    LAST_PROFILE.clear()
    LAST_PROFILE['exec_time_ns'] = res.exec_time_ns
    LAST_PROFILE['results'] = res
    return np.asarray(res.results[0]["out"], np.float32)
